# revision 1
# baseline (speedup 1.0000x reference)
"""PreconditionerSparseUNet on 8 TRN2 NeuronCores.

Sharding: data-parallel over batch (8 images, 1 per core). Each core runs the
full U-Net on its own 512x512x1 image; weights are replicated.

Per-core implementation notes:
- Feature maps live in DRAM as [C, flat] with flat = padded row-major spatial
  (Wp = W+2 with a zero ring) plus a guard margin at both ends so halo loads
  never leave the buffer.
- 3x3 convs are computed as a few matmuls per output chunk: input channel
  blocks are replicated into SBUF partitions at shifted offsets (K-folding),
  so one matmul contracts several taps at once. rhs/lhsT are fp16 (full PE
  rate), PSUM accumulates fp32.
- Conv outputs of M<=64 channels are packed 4x/2x along PSUM partitions so
  the bias+LeakyReLU epilogue runs once per 128-partition tile.
- Transposed convs (stride-2 upsampling) are decomposed into 4 output-parity
  classes, each a small conv over the input grid, written out with stride-2
  access patterns. The skip-add happens after the LeakyReLU, before store.
- Final 1x1 conv applies the triangular mask via a constant lower-tri
  multiplier; the diagonal is patched with softplus afterwards.
"""

import os

import numpy as np

import concourse.bass as bass
import concourse.bacc as bacc
import concourse.mybir as mybir
from concourse.tile import TileContext
from concourse.bass_utils import run_bass_kernel_spmd

AF = mybir.ActivationFunctionType
F32 = mybir.dt.float32
F16 = mybir.dt.float16

# Feature-map streaming dtype. fp16 halves DMA traffic; PSUM accumulation
# stays fp32. Flip both to float32 if accuracy ever demands it.
STREAM_DT = F16
STREAM_NP = np.float16

N = 512
B = 8
ALPHA = 0.01
GUARD = 1040  # >= 2*Wp+4 at the largest level (enc1 9-block shifts)

CH = [1, 16, 32, 64, 128, 1]


def wp(w):
    return w + 2


def buf_flat(w):
    return wp(w) * wp(w) + 2 * GUARD


# ----------------------------------------------------------------------------
# Matmul plans. Each matmul: dict(p0, segs, r) where segs is a list of
# (ky, kx) weight-slice picks (or None for zero rows), one per Cin-sized
# block slot starting at partition p0; r is the rhs offset in input-flat
# elements relative to the chunk base.
# ----------------------------------------------------------------------------

def s1_plan(cin, w):
    W = wp(w)
    if cin == 1:
        blocks = [ky * W + kx for ky in range(3) for kx in range(3)]
        mms = [dict(p0=0, segs=[(ky, kx) for ky in range(3) for kx in range(3)], r=0)]
    elif cin <= 32:
        blocks = [0, 1, 2]
        mms = [dict(p0=0, segs=[(ky, 0), (ky, 1), (ky, 2)], r=ky * W)
               for ky in range(3)]
    elif cin == 64:
        blocks = [0, 1]
        mms = []
        for ky in range(3):
            mms.append(dict(p0=0, segs=[(ky, 0), (ky, 1)], r=ky * W))
            mms.append(dict(p0=0, segs=[(ky, 2)], r=ky * W + 2))
    else:
        raise ValueError(cin)
    return blocks, mms


def s2_plan(cin, w_in):
    # identical tap algebra to s1 (offsets are in input-flat space)
    return s1_plan(cin, w_in)


def pmap(parity, d):
    # transposed conv: output parity p, input-tap delta d -> kernel index
    if parity == 0:
        return 1 if d == 0 else None
    return 0 if d == 0 else 2


def tconv_plan(cin, w_in):
    W = wp(w_in)
    if cin == 128:
        blocks = [0]
    elif cin == 64:
        blocks = [0, 1]  # dj shifts
    elif cin == 32:
        blocks = [0, 1, W, W + 1]  # (di,dj) = (0,0),(0,1),(1,0),(1,1)
    else:
        raise ValueError(cin)
    classes = []
    for py in range(2):
        for px in range(2):
            dis = [d for d in range(2) if pmap(py, d) is not None]
            djs = [d for d in range(2) if pmap(px, d) is not None]
            mms = []
            if cin == 128:
                for di in dis:
                    for dj in djs:
                        mms.append(dict(p0=0, segs=[(pmap(py, di), pmap(px, dj))],
                                        r=di * W + dj))
            elif cin == 64:
                for di in dis:
                    if len(djs) == 2:
                        mms.append(dict(p0=0,
                                        segs=[(pmap(py, di), pmap(px, 0)),
                                              (pmap(py, di), pmap(px, 1))],
                                        r=di * W))
                    else:
                        mms.append(dict(p0=0, segs=[(pmap(py, di), 1)],
                                        r=di * W))
            else:  # cin == 32, all four blocks resident
                if py == 0 and px == 0:
                    mms = [dict(p0=0, segs=[(1, 1)], r=0)]
                elif py == 0 and px == 1:
                    mms = [dict(p0=0, segs=[(1, 0), (1, 2)], r=0)]
                elif py == 1 and px == 0:
                    # one K=128 matmul over all four blocks; blocks holding
                    # dj=1 shifts get zero weight rows
                    mms = [dict(p0=0, segs=[(0, 1), None, (2, 1), None], r=0)]
                else:
                    mms = [dict(p0=0, segs=[(0, 0), (0, 2), (2, 0), (2, 2)], r=0)]
            classes.append((py, px, mms))
    return blocks, classes


# Layer table: (name, kind, cin, cout, w_in, w_out, wsrc, in, out, skip)
LAYERS = [
    ("enc1", "s1", 1, 16, 512, 512, "w_enc1", "xp", "enc1p", None),
    ("down1", "s2", 16, 32, 512, 256, "w_down1", "enc1p", "down1p", None),
    ("enc2", "s1", 32, 32, 256, 256, "w_enc2", "down1p", "enc2p", None),
    ("down2", "s2", 32, 64, 256, 128, "w_down2", "enc2p", "down2p", None),
    ("enc3", "s1", 64, 64, 128, 128, "w_enc3", "down2p", "enc3p", None),
    ("bn", "s2", 64, 128, 128, 64, "w_bn", "enc3p", "bnp", None),
    ("up2", "tc", 128, 64, 64, 128, "w_up2", "bnp", "up2p", "enc3p"),
    ("dec2", "s1", 64, 64, 128, 128, "w_dec2", "up2p", "dec2p", None),
    ("up1", "tc", 64, 32, 128, 256, "w_up1", "dec2p", "up1p", "enc2p"),
    ("dec1", "s1", 32, 32, 256, 256, "w_dec1", "up1p", "dec1p", None),
    ("up0", "tc", 32, 16, 256, 512, "w_up0", "dec1p", "up0p", "enc1p"),
    ("dec0", "s1", 16, 16, 512, 512, "w_dec0", "up0p", "dec0p", None),
]

# group sizes (pixels for s1, input/output rows for s2/tc)
GROUP = {
    "enc1": 8192, "down1": 8, "enc2": 8192, "down2": 8, "enc3": 8192,
    "bn": 8, "up2": 64, "dec2": 8192, "up1": 32, "dec1": 8192,
    "up0": 16, "dec0": 8192, "out": 16,
}


def pack_stride(cout):
    return 32 if cout <= 32 else (64 if cout == 64 else 128)


def layer_plan(kind, cin, w_in):
    if kind == "s1":
        return s1_plan(cin, w_in)
    if kind == "s2":
        return s2_plan(cin, w_in)
    return tconv_plan(cin, w_in)


def mm_keys(name, kind, cin, w_in):
    """Enumerate (key, p0, nseg) for every matmul of a layer."""
    out = []
    if kind in ("s1", "s2"):
        _, mms = layer_plan(kind, cin, w_in)
        for i, m in enumerate(mms):
            out.append((f"W_{name}_{i}", m))
    else:
        _, classes = layer_plan(kind, cin, w_in)
        for py, px, mms in classes:
            for i, m in enumerate(mms):
                out.append((f"W_{name}_c{py}{px}_{i}", m))
    return out


# ----------------------------------------------------------------------------
# Host-side input prep
# ----------------------------------------------------------------------------

def prep_weights(inputs):
    """Build per-matmul lhsT arrays (fp16) and packed bias vectors."""
    wmap = {}
    for (name, kind, cin, cout, w_in, w_out, wsrc, *_rest) in LAYERS:
        w = np.asarray(inputs[wsrc])  # [3,3,cin,cout]
        for key, m in mm_keys(name, kind, cin, w_in):
            segs = []
            for s in m["segs"]:
                if s is None:
                    segs.append(np.zeros((cin, cout), np.float32))
                else:
                    segs.append(w[s[0], s[1]])
            wmap[key] = np.ascontiguousarray(
                np.concatenate(segs, axis=0).astype(STREAM_NP))
        bsrc = "b_" + wsrc[2:]
        b = np.asarray(inputs[bsrc]).astype(np.float32)
        stride = pack_stride(cout)
        b128 = np.zeros((128, 1), np.float32)
        for pk in range(128 // stride):
            b128[pk * stride: pk * stride + cout, 0] = b
        wmap[f"B_{name}"] = b128
    # final 1x1 conv
    wmap["W_out"] = np.ascontiguousarray(
        np.asarray(inputs["w_out"]).reshape(16, 1).astype(STREAM_NP))
    wmap["B_out"] = np.full((128, 1), float(np.asarray(inputs["b_out"])[0]),
                            np.float32)
    return wmap


# ----------------------------------------------------------------------------
# Kernel builder
# ----------------------------------------------------------------------------

def sub_ap(base_ap, p0, np_, off, dims):
    """AP over partitions [p0,p0+np_) of base_ap with free dims at elem off."""
    pitch = base_ap.ap[0][0]
    return bass.AP(base_ap.tensor, base_ap.offset + p0 * pitch + off,
                   [[pitch, np_]] + [list(d) for d in dims])


def strided_part_ap(base_ap, p0, pstep, np_, off, dims):
    """AP over partitions p0, p0+pstep, ... of base_ap."""
    pitch = base_ap.ap[0][0]
    return bass.AP(base_ap.tensor, base_ap.offset + p0 * pitch + off,
                   [[pstep * pitch, np_]] + [list(d) for d in dims])


def build_unet():
    nc = bacc.Bacc("TRN2", target_bir_lowering=False, debug=False)

    # --- DRAM tensors -------------------------------------------------------
    x_in = nc.dram_tensor("x", [N * N], STREAM_DT, kind="ExternalInput").ap()
    out_t = nc.dram_tensor("out", [N, N], F32, kind="ExternalOutput").ap()

    bufs = {}
    for nm, w in [("xp", 512), ("enc1p", 512), ("down1p", 256), ("enc2p", 256),
                  ("down2p", 128), ("enc3p", 128), ("bnp", 64), ("up2p", 128),
                  ("dec2p", 128), ("up1p", 256), ("dec1p", 256), ("up0p", 512),
                  ("dec0p", 512)]:
        c = {"xp": 1, "enc1p": 16, "down1p": 32, "enc2p": 32, "down2p": 64,
             "enc3p": 64, "bnp": 128, "up2p": 64, "dec2p": 64, "up1p": 32,
             "dec1p": 32, "up0p": 16, "dec0p": 16}[nm]
        bufs[nm] = nc.dram_tensor(nm, [c, buf_flat(w)], STREAM_DT,
                                  kind="Internal").ap()

    win = {}
    for (name, kind, cin, cout, w_in, *_r) in LAYERS:
        for key, m in mm_keys(name, kind, cin, w_in):
            k = len(m["segs"]) * cin
            win[key] = nc.dram_tensor(key, [k, cout], STREAM_DT,
                                      kind="ExternalInput").ap()
        win[f"B_{name}"] = nc.dram_tensor(f"B_{name}", [128, 1], F32,
                                          kind="ExternalInput").ap()
    win["W_out"] = nc.dram_tensor("W_out", [16, 1], STREAM_DT,
                                  kind="ExternalInput").ap()
    win["B_out"] = nc.dram_tensor("B_out", [128, 1], F32,
                                  kind="ExternalInput").ap()

    mask_np = np.tril(np.ones((N, N), np.float32))
    mask_t = nc.inline_tensor(mask_np, name="trimask").ap()

    with TileContext(nc) as tc:
        with (
            tc.tile_pool(name="wpool", bufs=1) as wpool,
            tc.tile_pool(name="inpool", bufs=3) as inpool,
            tc.tile_pool(name="actpool", bufs=4) as actpool,
            tc.tile_pool(name="skpool", bufs=3) as skpool,
            tc.tile_pool(name="psum", bufs=8, space="PSUM") as pspool,
        ):
            # --- persistent tiles: weights, biases, zeros ------------------
            wt = {}
            for (name, kind, cin, cout, w_in, *_r) in LAYERS:
                for key, m in mm_keys(name, kind, cin, w_in):
                    k = len(m["segs"]) * cin
                    t = wpool.tile([128, cout], STREAM_DT, tag=key)
                    nc.sync.dma_start(out=t[m["p0"]:m["p0"] + k, :],
                                      in_=win[key])
                    wt[key] = t
                t = wpool.tile([128, 1], F32, tag=f"B_{name}")
                nc.sync.dma_start(out=t[:, :], in_=win[f"B_{name}"])
                wt[f"B_{name}"] = t
            t = wpool.tile([128, 1], STREAM_DT, tag="W_out")
            nc.sync.dma_start(out=t[0:16, :], in_=win["W_out"])
            wt["W_out"] = t
            t = wpool.tile([128, 1], F32, tag="B_out")
            nc.sync.dma_start(out=t[:, :], in_=win["B_out"])
            wt["B_out"] = t

            zt = wpool.tile([128, GUARD], STREAM_DT, tag="zeros")
            nc.any.memset(zt[:, :], 0.0)
            zt_ap = zt[:, :]

            def zero_ring(nm, c, w):
                bap = bufs[nm]
                W, H = wp(w), wp(w)
                nc.sync.dma_start(out=sub_ap(bap, 0, c, 0, [[1, GUARD]]),
                                  in_=zt_ap[0:c, 0:GUARD])
                nc.sync.dma_start(
                    out=sub_ap(bap, 0, c, GUARD + W * H, [[1, GUARD]]),
                    in_=zt_ap[0:c, 0:GUARD])
                nc.sync.dma_start(out=sub_ap(bap, 0, c, GUARD, [[1, W]]),
                                  in_=zt_ap[0:c, 0:W])
                nc.sync.dma_start(
                    out=sub_ap(bap, 0, c, GUARD + (H - 1) * W, [[1, W]]),
                    in_=zt_ap[0:c, 0:W])
                zero_cols(nm, c, w)

            def zero_cols(nm, c, w):
                bap = bufs[nm]
                W, H = wp(w), wp(w)
                nc.sync.dma_start(out=sub_ap(bap, 0, c, GUARD, [[W, H]]),
                                  in_=zt_ap[0:c, 0:H])
                nc.sync.dma_start(
                    out=sub_ap(bap, 0, c, GUARD + W - 1, [[W, H]]),
                    in_=zt_ap[0:c, 0:H])

            chans = {"xp": 1, "enc1p": 16, "down1p": 32, "enc2p": 32,
                     "down2p": 64, "enc3p": 64, "bnp": 128, "up2p": 64,
                     "dec2p": 64, "up1p": 32, "dec1p": 32, "up0p": 16,
                     "dec0p": 16}
            widths = {"xp": 512, "enc1p": 512, "down1p": 256, "enc2p": 256,
                      "down2p": 128, "enc3p": 128, "bnp": 64, "up2p": 128,
                      "dec2p": 128, "up1p": 256, "dec1p": 256, "up0p": 512,
                      "dec0p": 512}
            for nm in bufs:
                zero_ring(nm, chans[nm], widths[nm])

            # xp interior fill from x input
            Wx = wp(512)
            nc.sync.dma_start(
                out=sub_ap(bufs["xp"], 0, 1, GUARD + Wx + 1, [[Wx, 512], [1, 512]]),
                in_=x_in.rearrange("(h w) -> h w", w=512).unsqueeze(0))

            # --- layer emitters -------------------------------------------
            def emit_s1(name, cin, cout, w, inb, outb):
                W = wp(w)
                blocks, mms = s1_plan(cin, w)
                nb = len(blocks)
                stride = pack_stride(cout)
                pack = 128 // stride
                per_tile = 512 * pack
                total = w * W
                o_base = GUARD + W
                bias = wt[f"B_{name}"][:, :]
                G = GROUP[name]
                g = 0
                while g < total:
                    Gp = min(G, total - g)
                    span = Gp + 2 * W + 2
                    A0 = o_base + g - W - 1
                    tin = inpool.tile([nb * cin, span], STREAM_DT, tag="inb")
                    tin_ap = tin[:, :]
                    for j, s in enumerate(blocks):
                        nc.sync.dma_start(
                            out=tin[j * cin:(j + 1) * cin, :],
                            in_=sub_ap(bufs[inb], 0, cin, A0 + s, [[1, span]]))
                    t = 0
                    while t < Gp:
                        Tp = min(per_tile, Gp - t)
                        ps = pspool.tile([128, 512], F32, tag="ps")
                        nch = (Tp + 511) // 512
                        for pk in range(nch):
                            cn = min(512, Tp - pk * 512)
                            col = pk * stride
                            for mi, m in enumerate(mms):
                                K = len(m["segs"]) * cin
                                rhs = sub_ap(tin_ap, m["p0"], K,
                                             t + pk * 512 + m["r"], [[1, cn]])
                                nc.tensor.matmul(
                                    ps[col:col + cout, 0:cn],
                                    lhsT=wt[f"W_{name}_{mi}"][m["p0"]:m["p0"] + K, 0:cout],
                                    rhs=rhs,
                                    start=(mi == 0), stop=(mi == len(mms) - 1),
                                    tile_position=(m["p0"], col))
                        act = actpool.tile([128, 512], STREAM_DT, tag="act")
                        nc.scalar.activation(act[:, :], ps[:, :], AF.Prelu,
                                             bias=bias, alpha=ALPHA)
                        for pk in range(nch):
                            cn = min(512, Tp - pk * 512)
                            col = pk * stride
                            nc.sync.dma_start(
                                out=sub_ap(bufs[outb], 0, cout,
                                           o_base + g + t + pk * 512, [[1, cn]]),
                                in_=act[col:col + cout, 0:cn])
                        t += Tp
                    g += Gp
                zero_cols(outb, cout, w)

            def emit_s2(name, cin, cout, w_in, w_out, inb, outb):
                Wi, Wo = wp(w_in), wp(w_out)
                blocks, mms = s2_plan(cin, w_in)
                nb = len(blocks)
                stride = pack_stride(cout)
                pack = 128 // stride
                R = 512 // w_out          # out rows per chunk
                rows_pt = pack * R        # out rows per psum tile
                bias = wt[f"B_{name}"][:, :]
                Rg = GROUP[name]          # out rows per load group
                for y0 in range(0, w_out, Rg):
                    Ry = min(Rg, w_out - y0)
                    A0 = GUARD + 2 * y0 * Wi
                    span = (2 * Ry + 2) * Wi
                    tin = inpool.tile([nb * cin, span], STREAM_DT, tag="inb")
                    tin_ap = tin[:, :]
                    for j, s in enumerate(blocks):
                        nc.sync.dma_start(
                            out=tin[j * cin:(j + 1) * cin, :],
                            in_=sub_ap(bufs[inb], 0, cin, A0 + s, [[1, span]]))
                    yt = 0
                    while yt < Ry:
                        Rt = min(rows_pt, Ry - yt)
                        ps = pspool.tile([128, 512], F32, tag="ps")
                        nch = (Rt + R - 1) // R
                        for pk in range(nch):
                            rr = min(R, Rt - pk * R)
                            col = pk * stride
                            base = 2 * (yt + pk * R) * Wi
                            for mi, m in enumerate(mms):
                                K = len(m["segs"]) * cin
                                rhs = sub_ap(tin_ap, m["p0"], K, base + m["r"],
                                             [[2 * Wi, rr], [2, w_out]])
                                nc.tensor.matmul(
                                    ps[col:col + cout, 0:rr * w_out],
                                    lhsT=wt[f"W_{name}_{mi}"][m["p0"]:m["p0"] + K, 0:cout],
                                    rhs=rhs,
                                    start=(mi == 0), stop=(mi == len(mms) - 1),
                                    tile_position=(m["p0"], col))
                        act = actpool.tile([128, 512], STREAM_DT, tag="act")
                        act_ap = act[:, :]
                        nc.scalar.activation(act_ap, ps[:, :], AF.Prelu,
                                             bias=bias, alpha=ALPHA)
                        for pk in range(nch):
                            rr = min(R, Rt - pk * R)
                            col = pk * stride
                            yo = y0 + yt + pk * R
                            nc.sync.dma_start(
                                out=sub_ap(bufs[outb], 0, cout,
                                           GUARD + (1 + yo) * Wo + 1,
                                           [[Wo, rr], [1, w_out]]),
                                in_=sub_ap(act_ap, col, cout, 0,
                                           [[w_out, rr], [1, w_out]]))
                        yt += Rt
                    # no ring junk for s2 (interior only)

            def emit_tconv(name, cin, cout, w_in, w_out, inb, outb, skipb):
                Wi, Wo = wp(w_in), wp(w_out)
                blocks, classes = tconv_plan(cin, w_in)
                cls = {(py, px): mms for (py, px, mms) in classes}
                nb = len(blocks)
                stride = pack_stride(cout)
                pack = 128 // stride
                Ri = 512 // w_in          # input rows per chunk
                rows_pt = pack * Ri       # input rows per psum tile
                bias = wt[f"B_{name}"][:, :]
                Rg = GROUP[name]          # input rows per load group
                for i0 in range(0, w_in, Rg):
                    A0 = GUARD + (1 + i0) * Wi + 1
                    span = (Rg + 2) * Wi
                    tin = inpool.tile([nb * cin, span], STREAM_DT, tag="inb")
                    tin_ap = tin[:, :]
                    for j, s in enumerate(blocks):
                        nc.sync.dma_start(
                            out=tin[j * cin:(j + 1) * cin, :],
                            in_=sub_ap(bufs[inb], 0, cin, A0 + s, [[1, span]]))
                    for py in range(2):
                        for it in range(0, Rg, rows_pt):
                            ps_pair = []
                            for px in range(2):
                                mms = cls[(py, px)]
                                ps = pspool.tile([128, 512], F32, tag="ps")
                                for pk in range(pack):
                                    col = pk * stride
                                    base = (it + pk * Ri) * Wi
                                    for mi, m in enumerate(mms):
                                        K = len(m["segs"]) * cin
                                        rhs = sub_ap(tin_ap, m["p0"], K,
                                                     base + m["r"],
                                                     [[Wi, Ri], [1, w_in]])
                                        nc.tensor.matmul(
                                            ps[col:col + cout, 0:Ri * w_in],
                                            lhsT=wt[f"W_{name}_c{py}{px}_{mi}"][m["p0"]:m["p0"] + K, 0:cout],
                                            rhs=rhs,
                                            start=(mi == 0), stop=(mi == len(mms) - 1),
                                            tile_position=(m["p0"], col))
                                ps_pair.append(ps)
                            wide = actpool.tile([128, 1024], STREAM_DT, tag="wide")
                            wide_ap = wide[:, :]
                            wpitch = wide_ap.ap[0][0]
                            for px in range(2):
                                ps_ap = ps_pair[px][:, :]
                                ppitch = ps_ap.ap[0][0]
                                oap = bass.AP(wide_ap.tensor, wide_ap.offset + px,
                                              [[wpitch, 128], [2 * w_in, Ri], [2, w_in]])
                                iap = bass.AP(ps_ap.tensor, ps_ap.offset,
                                              [[ppitch, 128], [w_in, Ri], [1, w_in]])
                                nc.scalar.activation(oap, iap, AF.Prelu,
                                                     bias=bias, alpha=ALPHA)
                            skt = skpool.tile([128, 1024], STREAM_DT, tag="skt")
                            skt_ap = skt[:, :]
                            for pk in range(pack):
                                io = i0 + it + pk * Ri
                                off = GUARD + (1 + 2 * io + py) * Wo + 1
                                nc.sync.dma_start(
                                    out=sub_ap(skt_ap, pk * stride, cout, 0,
                                               [[2 * w_in, Ri], [1, 2 * w_in]]),
                                    in_=sub_ap(bufs[skipb], 0, cout, off,
                                               [[2 * Wo, Ri], [1, 2 * w_in]]))
                            nc.vector.tensor_add(out=wide_ap, in0=wide_ap,
                                                 in1=skt_ap)
                            for pk in range(pack):
                                io = i0 + it + pk * Ri
                                off = GUARD + (1 + 2 * io + py) * Wo + 1
                                nc.sync.dma_start(
                                    out=sub_ap(bufs[outb], 0, cout, off,
                                               [[2 * Wo, Ri], [1, 2 * w_in]]),
                                    in_=sub_ap(wide_ap, pk * stride, cout, 0,
                                               [[2 * w_in, Ri], [1, 2 * w_in]]))

            nlayers = int(os.environ.get("UNET_NLAYERS", "99"))
            for (name, kind, cin, cout, w_in, w_out, wsrc, inb, outb, skipb) in LAYERS[:nlayers]:
                if kind == "s1":
                    emit_s1(name, cin, cout, w_in, inb, outb)
                elif kind == "s2":
                    emit_s2(name, cin, cout, w_in, w_out, inb, outb)
                else:
                    emit_tconv(name, cin, cout, w_in, w_out, inb, outb, skipb)

            # --- final 1x1 conv + triangular masking ----------------------
            do_tail = nlayers > len(LAYERS)
            if not do_tail:
                nc.sync.dma_start(out=out_t[:, :], in_=mask_t[:, :])
            W0 = wp(512)
            if do_tail:
                bias = wt["B_out"][:, :]
                Rg = GROUP["out"]
                for y0 in range(0, 512, Rg):
                    A0 = GUARD + (1 + y0) * W0 + 1
                    span = (Rg - 1) * W0 + 512
                    tin = inpool.tile([16, span], STREAM_DT, tag="inb")
                    tin_ap = tin[:, :]
                    nc.sync.dma_start(out=tin[:, :],
                                      in_=sub_ap(bufs["dec0p"], 0, 16, A0, [[1, span]]))
                    for yt in range(0, Rg, 4):
                        ps = pspool.tile([128, 512], F32, tag="ps")
                        for pk in range(4):
                            rhs = sub_ap(tin_ap, 0, 16, (yt + pk) * W0, [[1, 512]])
                            nc.tensor.matmul(ps[pk * 32:pk * 32 + 1, :],
                                             lhsT=wt["W_out"][0:16, 0:1], rhs=rhs,
                                             start=True, stop=True,
                                             tile_position=(0, pk * 32))
                        act = actpool.tile([128, 512], F32, tag="actf")
                        nc.scalar.activation(act[:, :], ps[:, :], AF.Identity,
                                             bias=bias)
                        mt = skpool.tile([128, 512], F32, tag="mask")
                        nc.sync.dma_start(
                            out=strided_part_ap(mt[:, :], 0, 32, 4, 0, [[1, 512]]),
                            in_=mask_t[y0 + yt:y0 + yt + 4, :])
                        nc.vector.tensor_mul(out=act[:, :], in0=act[:, :],
                                             in1=mt[:, :])
                        nc.sync.dma_start(
                            out=out_t[y0 + yt:y0 + yt + 4, :],
                            in_=strided_part_ap(act[:, :], 0, 32, 4, 0, [[1, 512]]))

            # --- diagonal softplus patch ----------------------------------
            # softplus(x) = relu(x) + ln(1 + exp(-|x|)), built from table ops
            do_diag = nlayers > len(LAYERS) + 1
            if do_diag:
                out_flat = out_t.flatten()
                diag_ap = bass.AP(out_flat.tensor, out_flat.offset, [[513, 512]])
                dt_ = actpool.tile([1, 512], F32, tag="diag")
                nc.sync.dma_start(out=dt_[:, :], in_=diag_ap)
                ta = actpool.tile([1, 512], F32, tag="diag_a")
                nc.scalar.activation(ta[:, :], dt_[:, :], AF.Abs)
                nc.scalar.activation(ta[:, :], ta[:, :], AF.Exp, scale=-1.0)
                nc.vector.tensor_scalar_add(out=ta[:, :], in0=ta[:, :], scalar1=1.0)
                nc.scalar.activation(ta[:, :], ta[:, :], AF.Ln)
                tr = actpool.tile([1, 512], F32, tag="diag_r")
                nc.scalar.activation(tr[:, :], dt_[:, :], AF.Relu)
                nc.vector.tensor_add(out=tr[:, :], in0=tr[:, :], in1=ta[:, :])
                nc.sync.dma_start(out=diag_ap, in_=tr[:, :])

    nc.compile()
    return nc


_NC_CACHE = None


def get_nc():
    global _NC_CACHE
    if _NC_CACHE is None:
        _NC_CACHE = build_unet()
    return _NC_CACHE


def make_in_maps(inputs):
    wmap = prep_weights(inputs)
    x = np.asarray(inputs["x"])  # [8, 512, 512, 1] f32
    in_maps = []
    for i in range(B):
        m = dict(wmap)
        m["x"] = np.ascontiguousarray(
            x[i, :, :, 0].reshape(-1).astype(STREAM_NP))
        in_maps.append(m)
    return in_maps


def kernel(_trace=False, **inputs):
    nc = get_nc()
    in_maps = make_in_maps(inputs)
    res = run_bass_kernel_spmd(nc, in_maps, core_ids=list(range(B)),
                               trace=_trace)
    out = np.stack([res.results[i]["out"] for i in range(B)], axis=0)
    out = out[:, :, :, None].astype(np.float32)
    if _trace:
        return out, res
    return out



# revision 10
# speedup vs baseline: 1.0494x; 1.0494x over previous
"""PreconditionerSparseUNet on 8 TRN2 NeuronCores.

Sharding: data-parallel over batch (8 images, 1 per core). Each core runs the
full U-Net on its own 512x512x1 image; weights are replicated.

Per-core implementation notes:
- Feature maps live in DRAM as [C, flat] with flat = padded row-major spatial
  (Wp = W+2 with a zero ring) plus a guard margin at both ends so halo loads
  never leave the buffer.
- 3x3 convs are computed as a few matmuls per output chunk: input channel
  blocks are replicated into SBUF partitions at shifted offsets (K-folding),
  so one matmul contracts several taps at once. rhs/lhsT are fp16 (full PE
  rate), PSUM accumulates fp32.
- Conv outputs of M<=64 channels are packed 4x/2x along PSUM partitions so
  the bias+LeakyReLU epilogue runs once per 128-partition tile.
- Transposed convs (stride-2 upsampling) are decomposed into 4 output-parity
  classes, each a small conv over the input grid, written out with stride-2
  access patterns. The skip-add happens after the LeakyReLU, before store.
- Final 1x1 conv applies the triangular mask via a constant lower-tri
  multiplier; the diagonal is patched with softplus afterwards.
"""

import os

import numpy as np

import concourse.bass as bass
import concourse.bacc as bacc
import concourse.mybir as mybir
from concourse.tile import TileContext
from concourse.bass_utils import run_bass_kernel_spmd

AF = mybir.ActivationFunctionType
F32 = mybir.dt.float32
F16 = mybir.dt.float16

# Feature-map streaming dtype. fp16 halves DMA traffic; PSUM accumulation
# stays fp32. Flip both to float32 if accuracy ever demands it.
STREAM_DT = F16
STREAM_NP = np.float16

N = 512
B = 8
ALPHA = 0.01
GUARD = 1040  # >= 2*Wp+4 at the largest level (enc1 9-block shifts)

CH = [1, 16, 32, 64, 128, 1]


def wp(w):
    return w + 2


def buf_flat(w):
    return wp(w) * wp(w) + 2 * GUARD


# ----------------------------------------------------------------------------
# Matmul plans. Each matmul: dict(p0, segs, r) where segs is a list of
# (ky, kx) weight-slice picks (or None for zero rows), one per Cin-sized
# block slot starting at partition p0; r is the rhs offset in input-flat
# elements relative to the chunk base.
# ----------------------------------------------------------------------------

def s1_plan(cin, w):
    W = wp(w)
    if cin == 1:
        blocks = [ky * W + kx for ky in range(3) for kx in range(3)]
        mms = [dict(p0=0, segs=[(ky, kx) for ky in range(3) for kx in range(3)], r=0)]
    elif cin <= 32:
        blocks = [0, 1, 2]
        mms = [dict(p0=0, segs=[(ky, 0), (ky, 1), (ky, 2)], r=ky * W)
               for ky in range(3)]
    elif cin == 64:
        blocks = [0, 1]
        mms = []
        for ky in range(3):
            mms.append(dict(p0=0, segs=[(ky, 0), (ky, 1)], r=ky * W))
            mms.append(dict(p0=0, segs=[(ky, 2)], r=ky * W + 2))
    else:
        raise ValueError(cin)
    return blocks, mms


def s2_plan(cin, w_in):
    # identical tap algebra to s1 (offsets are in input-flat space)
    return s1_plan(cin, w_in)


def pmap(parity, d):
    # transposed conv: output parity p, input-tap delta d -> kernel index
    if parity == 0:
        return 1 if d == 0 else None
    return 0 if d == 0 else 2


def tconv_plan(cin, w_in):
    W = wp(w_in)
    if cin == 128:
        blocks = [0]
    elif cin == 64:
        blocks = [0, 1]  # dj shifts
    elif cin == 32:
        blocks = [0, 1, W, W + 1]  # (di,dj) = (0,0),(0,1),(1,0),(1,1)
    else:
        raise ValueError(cin)
    classes = []
    for py in range(2):
        for px in range(2):
            dis = [d for d in range(2) if pmap(py, d) is not None]
            djs = [d for d in range(2) if pmap(px, d) is not None]
            mms = []
            if cin == 128:
                for di in dis:
                    for dj in djs:
                        mms.append(dict(p0=0, segs=[(pmap(py, di), pmap(px, dj))],
                                        r=di * W + dj))
            elif cin == 64:
                for di in dis:
                    if len(djs) == 2:
                        mms.append(dict(p0=0,
                                        segs=[(pmap(py, di), pmap(px, 0)),
                                              (pmap(py, di), pmap(px, 1))],
                                        r=di * W))
                    else:
                        mms.append(dict(p0=0, segs=[(pmap(py, di), 1)],
                                        r=di * W))
            else:  # cin == 32, all four blocks resident
                if py == 0 and px == 0:
                    mms = [dict(p0=0, segs=[(1, 1)], r=0)]
                elif py == 0 and px == 1:
                    mms = [dict(p0=0, segs=[(1, 0), (1, 2)], r=0)]
                elif py == 1 and px == 0:
                    # one K=128 matmul over all four blocks; blocks holding
                    # dj=1 shifts get zero weight rows
                    mms = [dict(p0=0, segs=[(0, 1), None, (2, 1), None], r=0)]
                else:
                    mms = [dict(p0=0, segs=[(0, 0), (0, 2), (2, 0), (2, 2)], r=0)]
            classes.append((py, px, mms))
    return blocks, classes


# Layer table: (name, kind, cin, cout, w_in, w_out, wsrc, in, out, skip)
LAYERS = [
    ("enc1", "s1", 1, 16, 512, 512, "w_enc1", "xp", "enc1p", None),
    ("down1", "s2", 16, 32, 512, 256, "w_down1", "enc1p", "down1p", None),
    ("enc2", "s1", 32, 32, 256, 256, "w_enc2", "down1p", "enc2p", None),
    ("down2", "s2", 32, 64, 256, 128, "w_down2", "enc2p", "down2p", None),
    ("enc3", "s1", 64, 64, 128, 128, "w_enc3", "down2p", "enc3p", None),
    ("bn", "s2", 64, 128, 128, 64, "w_bn", "enc3p", "bnp", None),
    ("up2", "tc", 128, 64, 64, 128, "w_up2", "bnp", "up2p", "enc3p"),
    ("dec2", "s1", 64, 64, 128, 128, "w_dec2", "up2p", "dec2p", None),
    ("up1", "tc", 64, 32, 128, 256, "w_up1", "dec2p", "up1p", "enc2p"),
    ("dec1", "s1", 32, 32, 256, 256, "w_dec1", "up1p", "dec1p", None),
    ("up0", "tc", 32, 16, 256, 512, "w_up0", "dec1p", "up0p", "enc1p"),
    ("dec0", "s1", 16, 16, 512, 512, "w_dec0", "up0p", "dec0p", None),
]

# group sizes (pixels for s1, input/output rows for s2/tc)
GROUP = {
    "enc1": 8192, "down1": 8, "enc2": 8192, "down2": 8, "enc3": 8192,
    "bn": 8, "up2": 64, "dec2": 8192, "up1": 32, "dec1": 8192,
    "up0": 16, "dec0": 8192, "out": 16,
}


def pack_stride(cout):
    return 32 if cout <= 32 else (64 if cout == 64 else 128)


def layer_plan(kind, cin, w_in):
    if kind == "s1":
        return s1_plan(cin, w_in)
    if kind == "s2":
        return s2_plan(cin, w_in)
    return tconv_plan(cin, w_in)


def mm_keys(name, kind, cin, w_in):
    """Enumerate (key, p0, nseg) for every matmul of a layer."""
    out = []
    if kind in ("s1", "s2"):
        _, mms = layer_plan(kind, cin, w_in)
        for i, m in enumerate(mms):
            out.append((f"W_{name}_{i}", m))
    else:
        _, classes = layer_plan(kind, cin, w_in)
        for py, px, mms in classes:
            for i, m in enumerate(mms):
                out.append((f"W_{name}_c{py}{px}_{i}", m))
    return out


# ----------------------------------------------------------------------------
# Host-side input prep
# ----------------------------------------------------------------------------

def prep_weights(inputs):
    """Build per-matmul lhsT arrays (fp16) and packed bias vectors."""
    wmap = {}
    for (name, kind, cin, cout, w_in, w_out, wsrc, *_rest) in LAYERS:
        w = np.asarray(inputs[wsrc])  # [3,3,cin,cout]
        for key, m in mm_keys(name, kind, cin, w_in):
            segs = []
            for s in m["segs"]:
                if s is None:
                    segs.append(np.zeros((cin, cout), np.float32))
                else:
                    segs.append(w[s[0], s[1]])
            wmap[key] = np.ascontiguousarray(
                np.concatenate(segs, axis=0).astype(STREAM_NP))
        bsrc = "b_" + wsrc[2:]
        b = np.asarray(inputs[bsrc]).astype(np.float32)
        stride = pack_stride(cout)
        b128 = np.zeros((128, 1), np.float32)
        for pk in range(128 // stride):
            b128[pk * stride: pk * stride + cout, 0] = b
        wmap[f"B_{name}"] = b128
    # final 1x1 conv
    wmap["W_out"] = np.ascontiguousarray(
        np.asarray(inputs["w_out"]).reshape(16, 1).astype(STREAM_NP))
    wmap["B_out"] = np.full((128, 1), float(np.asarray(inputs["b_out"])[0]),
                            np.float32)
    return wmap


# ----------------------------------------------------------------------------
# Kernel builder
# ----------------------------------------------------------------------------

def sub_ap(base_ap, p0, np_, off, dims):
    """AP over partitions [p0,p0+np_) of base_ap with free dims at elem off."""
    pitch = base_ap.ap[0][0]
    return bass.AP(base_ap.tensor, base_ap.offset + p0 * pitch + off,
                   [[pitch, np_]] + [list(d) for d in dims])


def strided_part_ap(base_ap, p0, pstep, np_, off, dims):
    """AP over partitions p0, p0+pstep, ... of base_ap."""
    pitch = base_ap.ap[0][0]
    return bass.AP(base_ap.tensor, base_ap.offset + p0 * pitch + off,
                   [[pstep * pitch, np_]] + [list(d) for d in dims])


def build_unet():
    nc = bacc.Bacc("TRN2", target_bir_lowering=False, debug=False)

    # --- DRAM tensors -------------------------------------------------------
    x_in = nc.dram_tensor("x", [N * N], STREAM_DT, kind="ExternalInput").ap()
    out_t = nc.dram_tensor("out", [N, N], F32, kind="ExternalOutput").ap()

    bufs = {}
    for nm, w in [("xp", 512), ("enc1p", 512), ("down1p", 256), ("enc2p", 256),
                  ("down2p", 128), ("enc3p", 128), ("bnp", 64), ("up2p", 128),
                  ("dec2p", 128), ("up1p", 256), ("dec1p", 256), ("up0p", 512),
                  ("dec0p", 512)]:
        c = {"xp": 1, "enc1p": 16, "down1p": 32, "enc2p": 32, "down2p": 64,
             "enc3p": 64, "bnp": 128, "up2p": 64, "dec2p": 64, "up1p": 32,
             "dec1p": 32, "up0p": 16, "dec0p": 16}[nm]
        bufs[nm] = nc.dram_tensor(nm, [c, buf_flat(w)], STREAM_DT,
                                  kind="Internal").ap()

    win = {}
    for (name, kind, cin, cout, w_in, *_r) in LAYERS:
        for key, m in mm_keys(name, kind, cin, w_in):
            k = len(m["segs"]) * cin
            win[key] = nc.dram_tensor(key, [k, cout], STREAM_DT,
                                      kind="ExternalInput").ap()
        win[f"B_{name}"] = nc.dram_tensor(f"B_{name}", [128, 1], F32,
                                          kind="ExternalInput").ap()
    win["W_out"] = nc.dram_tensor("W_out", [16, 1], STREAM_DT,
                                  kind="ExternalInput").ap()
    win["B_out"] = nc.dram_tensor("B_out", [128, 1], F32,
                                  kind="ExternalInput").ap()

    mask_np = np.tril(np.ones((N, N), np.float32))
    mask_t = nc.inline_tensor(mask_np, name="trimask").ap()

    with TileContext(nc) as tc:
        with (
            tc.tile_pool(name="wpool", bufs=1) as wpool,
            tc.tile_pool(name="inpool", bufs=3) as inpool,
            tc.tile_pool(name="actpool", bufs=4) as actpool,
            tc.tile_pool(name="skpool", bufs=3) as skpool,
            tc.tile_pool(name="psum", bufs=8, space="PSUM") as pspool,
        ):
            # --- persistent tiles: weights, biases, zeros ------------------
            wt = {}
            for (name, kind, cin, cout, w_in, *_r) in LAYERS:
                for key, m in mm_keys(name, kind, cin, w_in):
                    k = len(m["segs"]) * cin
                    t = wpool.tile([128, cout], STREAM_DT, tag=key)
                    nc.sync.dma_start(out=t[m["p0"]:m["p0"] + k, :],
                                      in_=win[key])
                    wt[key] = t
                t = wpool.tile([128, 1], F32, tag=f"B_{name}")
                nc.sync.dma_start(out=t[:, :], in_=win[f"B_{name}"])
                wt[f"B_{name}"] = t
            t = wpool.tile([128, 1], STREAM_DT, tag="W_out")
            nc.sync.dma_start(out=t[0:16, :], in_=win["W_out"])
            wt["W_out"] = t
            t = wpool.tile([128, 1], F32, tag="B_out")
            nc.sync.dma_start(out=t[:, :], in_=win["B_out"])
            wt["B_out"] = t

            zt = wpool.tile([128, GUARD], STREAM_DT, tag="zeros")
            nc.any.memset(zt[:, :], 0.0)
            zt_ap = zt[:, :]

            def zero_ring(nm, c, w):
                bap = bufs[nm]
                W, H = wp(w), wp(w)
                nc.sync.dma_start(out=sub_ap(bap, 0, c, 0, [[1, GUARD]]),
                                  in_=zt_ap[0:c, 0:GUARD])
                nc.sync.dma_start(
                    out=sub_ap(bap, 0, c, GUARD + W * H, [[1, GUARD]]),
                    in_=zt_ap[0:c, 0:GUARD])
                nc.sync.dma_start(out=sub_ap(bap, 0, c, GUARD, [[1, W]]),
                                  in_=zt_ap[0:c, 0:W])
                nc.sync.dma_start(
                    out=sub_ap(bap, 0, c, GUARD + (H - 1) * W, [[1, W]]),
                    in_=zt_ap[0:c, 0:W])
                zero_cols(nm, c, w)

            def zero_cols(nm, c, w):
                bap = bufs[nm]
                W, H = wp(w), wp(w)
                nc.sync.dma_start(out=sub_ap(bap, 0, c, GUARD, [[W, H]]),
                                  in_=zt_ap[0:c, 0:H])
                nc.sync.dma_start(
                    out=sub_ap(bap, 0, c, GUARD + W - 1, [[W, H]]),
                    in_=zt_ap[0:c, 0:H])

            chans = {"xp": 1, "enc1p": 16, "down1p": 32, "enc2p": 32,
                     "down2p": 64, "enc3p": 64, "bnp": 128, "up2p": 64,
                     "dec2p": 64, "up1p": 32, "dec1p": 32, "up0p": 16,
                     "dec0p": 16}
            widths = {"xp": 512, "enc1p": 512, "down1p": 256, "enc2p": 256,
                      "down2p": 128, "enc3p": 128, "bnp": 64, "up2p": 128,
                      "dec2p": 128, "up1p": 256, "dec1p": 256, "up0p": 512,
                      "dec0p": 512}
            for nm in bufs:
                zero_ring(nm, chans[nm], widths[nm])

            # xp interior fill from x input
            Wx = wp(512)
            nc.sync.dma_start(
                out=sub_ap(bufs["xp"], 0, 1, GUARD + Wx + 1, [[Wx, 512], [1, 512]]),
                in_=x_in.rearrange("(h w) -> h w", w=512).unsqueeze(0))

            # --- layer emitters -------------------------------------------
            def emit_s1(name, cin, cout, w, inb, outb):
                W = wp(w)
                blocks, mms = s1_plan(cin, w)
                nb = len(blocks)
                stride = pack_stride(cout)
                pack = 128 // stride
                per_tile = 512 * pack
                total = w * W
                o_base = GUARD + W
                bias = wt[f"B_{name}"][:, :]
                G = GROUP[name]
                g = 0
                while g < total:
                    Gp = min(G, total - g)
                    span = Gp + 2 * W + 2
                    A0 = o_base + g - W - 1
                    tin = inpool.tile([nb * cin, span], STREAM_DT, tag="inb")
                    tin_ap = tin[:, :]
                    for j, s in enumerate(blocks):
                        nc.sync.dma_start(
                            out=tin[j * cin:(j + 1) * cin, :],
                            in_=sub_ap(bufs[inb], 0, cin, A0 + s, [[1, span]]))
                    t = 0
                    while t < Gp:
                        Tp = min(per_tile, Gp - t)
                        ps = pspool.tile([128, 512], F32, tag="ps")
                        nch = (Tp + 511) // 512
                        for pk in range(nch):
                            cn = min(512, Tp - pk * 512)
                            col = pk * stride
                            for mi, m in enumerate(mms):
                                K = len(m["segs"]) * cin
                                rhs = sub_ap(tin_ap, m["p0"], K,
                                             t + pk * 512 + m["r"], [[1, cn]])
                                nc.tensor.matmul(
                                    ps[col:col + cout, 0:cn],
                                    lhsT=wt[f"W_{name}_{mi}"][m["p0"]:m["p0"] + K, 0:cout],
                                    rhs=rhs,
                                    start=(mi == 0), stop=(mi == len(mms) - 1),
                                    tile_position=(m["p0"], col))
                        act = actpool.tile([128, 512], STREAM_DT, tag="act")
                        nc.scalar.activation(act[:, :], ps[:, :], AF.Prelu,
                                             bias=bias, alpha=ALPHA)
                        for pk in range(nch):
                            cn = min(512, Tp - pk * 512)
                            col = pk * stride
                            nc.sync.dma_start(
                                out=sub_ap(bufs[outb], 0, cout,
                                           o_base + g + t + pk * 512, [[1, cn]]),
                                in_=act[col:col + cout, 0:cn])
                        t += Tp
                    g += Gp
                zero_cols(outb, cout, w)

            def emit_s2(name, cin, cout, w_in, w_out, inb, outb):
                Wi, Wo = wp(w_in), wp(w_out)
                blocks, mms = s2_plan(cin, w_in)
                nb = len(blocks)
                stride = pack_stride(cout)
                pack = 128 // stride
                R = 512 // w_out          # out rows per chunk
                rows_pt = pack * R        # out rows per psum tile
                bias = wt[f"B_{name}"][:, :]
                Rg = GROUP[name]          # out rows per load group
                for y0 in range(0, w_out, Rg):
                    Ry = min(Rg, w_out - y0)
                    A0 = GUARD + 2 * y0 * Wi
                    span = (2 * Ry + 2) * Wi
                    tin = inpool.tile([nb * cin, span], STREAM_DT, tag="inb")
                    tin_ap = tin[:, :]
                    for j, s in enumerate(blocks):
                        nc.sync.dma_start(
                            out=tin[j * cin:(j + 1) * cin, :],
                            in_=sub_ap(bufs[inb], 0, cin, A0 + s, [[1, span]]))
                    yt = 0
                    while yt < Ry:
                        Rt = min(rows_pt, Ry - yt)
                        ps = pspool.tile([128, 512], F32, tag="ps")
                        nch = (Rt + R - 1) // R
                        for pk in range(nch):
                            rr = min(R, Rt - pk * R)
                            col = pk * stride
                            base = 2 * (yt + pk * R) * Wi
                            for mi, m in enumerate(mms):
                                K = len(m["segs"]) * cin
                                rhs = sub_ap(tin_ap, m["p0"], K, base + m["r"],
                                             [[2 * Wi, rr], [2, w_out]])
                                nc.tensor.matmul(
                                    ps[col:col + cout, 0:rr * w_out],
                                    lhsT=wt[f"W_{name}_{mi}"][m["p0"]:m["p0"] + K, 0:cout],
                                    rhs=rhs,
                                    start=(mi == 0), stop=(mi == len(mms) - 1),
                                    tile_position=(m["p0"], col))
                        act = actpool.tile([128, 512], STREAM_DT, tag="act")
                        act_ap = act[:, :]
                        nc.scalar.activation(act_ap, ps[:, :], AF.Prelu,
                                             bias=bias, alpha=ALPHA)
                        for pk in range(nch):
                            rr = min(R, Rt - pk * R)
                            col = pk * stride
                            yo = y0 + yt + pk * R
                            nc.sync.dma_start(
                                out=sub_ap(bufs[outb], 0, cout,
                                           GUARD + (1 + yo) * Wo + 1,
                                           [[Wo, rr], [1, w_out]]),
                                in_=sub_ap(act_ap, col, cout, 0,
                                           [[w_out, rr], [1, w_out]]))
                        yt += Rt
                    # no ring junk for s2 (interior only)

            def emit_tconv(name, cin, cout, w_in, w_out, inb, outb, skipb):
                Wi, Wo = wp(w_in), wp(w_out)
                blocks, classes = tconv_plan(cin, w_in)
                cls = {(py, px): mms for (py, px, mms) in classes}
                nb = len(blocks)
                stride = pack_stride(cout)
                pack = 128 // stride
                Ri = 512 // w_in          # input rows per chunk
                rows_pt = pack * Ri       # input rows per psum tile
                bias = wt[f"B_{name}"][:, :]
                Rg = GROUP[name]          # input rows per load group
                for i0 in range(0, w_in, Rg):
                    A0 = GUARD + (1 + i0) * Wi + 1
                    span = (Rg + 2) * Wi
                    tin = inpool.tile([nb * cin, span], STREAM_DT, tag="inb")
                    tin_ap = tin[:, :]
                    for j, s in enumerate(blocks):
                        nc.sync.dma_start(
                            out=tin[j * cin:(j + 1) * cin, :],
                            in_=sub_ap(bufs[inb], 0, cin, A0 + s, [[1, span]]))
                    for py in range(2):
                        for it in range(0, Rg, rows_pt):
                            ps_pair = []
                            for px in range(2):
                                mms = cls[(py, px)]
                                ps = pspool.tile([128, 512], F32, tag="ps")
                                for pk in range(pack):
                                    col = pk * stride
                                    base = (it + pk * Ri) * Wi
                                    for mi, m in enumerate(mms):
                                        K = len(m["segs"]) * cin
                                        rhs = sub_ap(tin_ap, m["p0"], K,
                                                     base + m["r"],
                                                     [[Wi, Ri], [1, w_in]])
                                        nc.tensor.matmul(
                                            ps[col:col + cout, 0:Ri * w_in],
                                            lhsT=wt[f"W_{name}_c{py}{px}_{mi}"][m["p0"]:m["p0"] + K, 0:cout],
                                            rhs=rhs,
                                            start=(mi == 0), stop=(mi == len(mms) - 1),
                                            tile_position=(m["p0"], col))
                                ps_pair.append(ps)
                            wide = actpool.tile([128, 1024], STREAM_DT, tag="wide")
                            wide_ap = wide[:, :]
                            wpitch = wide_ap.ap[0][0]
                            for px in range(2):
                                ps_ap = ps_pair[px][:, :]
                                ppitch = ps_ap.ap[0][0]
                                oap = bass.AP(wide_ap.tensor, wide_ap.offset + px,
                                              [[wpitch, 128], [2 * w_in, Ri], [2, w_in]])
                                iap = bass.AP(ps_ap.tensor, ps_ap.offset,
                                              [[ppitch, 128], [w_in, Ri], [1, w_in]])
                                nc.scalar.activation(oap, iap, AF.Prelu,
                                                     bias=bias, alpha=ALPHA)
                            skt = skpool.tile([128, 1024], STREAM_DT, tag="skt")
                            skt_ap = skt[:, :]
                            for pk in range(pack):
                                io = i0 + it + pk * Ri
                                off = GUARD + (1 + 2 * io + py) * Wo + 1
                                nc.sync.dma_start(
                                    out=sub_ap(skt_ap, pk * stride, cout, 0,
                                               [[2 * w_in, Ri], [1, 2 * w_in]]),
                                    in_=sub_ap(bufs[skipb], 0, cout, off,
                                               [[2 * Wo, Ri], [1, 2 * w_in]]))
                            nc.vector.tensor_add(out=wide_ap, in0=wide_ap,
                                                 in1=skt_ap)
                            for pk in range(pack):
                                io = i0 + it + pk * Ri
                                off = GUARD + (1 + 2 * io + py) * Wo + 1
                                nc.sync.dma_start(
                                    out=sub_ap(bufs[outb], 0, cout, off,
                                               [[2 * Wo, Ri], [1, 2 * w_in]]),
                                    in_=sub_ap(wide_ap, pk * stride, cout, 0,
                                               [[2 * w_in, Ri], [1, 2 * w_in]]))

            nlayers = int(os.environ.get("UNET_NLAYERS", "99"))
            for (name, kind, cin, cout, w_in, w_out, wsrc, inb, outb, skipb) in LAYERS[:nlayers]:
                if kind == "s1":
                    emit_s1(name, cin, cout, w_in, inb, outb)
                elif kind == "s2":
                    emit_s2(name, cin, cout, w_in, w_out, inb, outb)
                else:
                    emit_tconv(name, cin, cout, w_in, w_out, inb, outb, skipb)

            # --- final 1x1 conv + triangular masking ----------------------
            do_tail = nlayers > len(LAYERS)
            if not do_tail:
                nc.sync.dma_start(out=out_t[:, :], in_=mask_t[:, :])
            W0 = wp(512)
            if do_tail:
                bias = wt["B_out"][:, :]
                Rg = GROUP["out"]
                for y0 in range(0, 512, Rg):
                    A0 = GUARD + (1 + y0) * W0 + 1
                    span = (Rg - 1) * W0 + 512
                    tin = inpool.tile([16, span], STREAM_DT, tag="inb")
                    tin_ap = tin[:, :]
                    nc.sync.dma_start(out=tin[:, :],
                                      in_=sub_ap(bufs["dec0p"], 0, 16, A0, [[1, span]]))
                    for yt in range(0, Rg, 4):
                        ps = pspool.tile([128, 512], F32, tag="ps")
                        for pk in range(4):
                            rhs = sub_ap(tin_ap, 0, 16, (yt + pk) * W0, [[1, 512]])
                            nc.tensor.matmul(ps[pk * 32:pk * 32 + 1, :],
                                             lhsT=wt["W_out"][0:16, 0:1], rhs=rhs,
                                             start=True, stop=True,
                                             tile_position=(0, pk * 32))
                        act = actpool.tile([128, 512], F32, tag="actf")
                        nc.scalar.activation(act[:, :], ps[:, :], AF.Identity,
                                             bias=bias)
                        mt = skpool.tile([128, 512], F32, tag="mask")
                        nc.sync.dma_start(
                            out=strided_part_ap(mt[:, :], 0, 32, 4, 0, [[1, 512]]),
                            in_=mask_t[y0 + yt:y0 + yt + 4, :])
                        nc.vector.tensor_mul(out=act[:, :], in0=act[:, :],
                                             in1=mt[:, :])
                        nc.sync.dma_start(
                            out=out_t[y0 + yt:y0 + yt + 4, :],
                            in_=strided_part_ap(act[:, :], 0, 32, 4, 0, [[1, 512]]))

            # --- diagonal softplus patch ----------------------------------
            # softplus(x) = relu(x) + ln(1 + exp(-|x|)), built from table ops
            do_diag = nlayers > len(LAYERS) + 1
            if do_diag:
                out_flat = out_t.flatten()
                diag_ap = bass.AP(out_flat.tensor, out_flat.offset, [[513, 512]])
                dt_ = actpool.tile([1, 512], F32, tag="diag")
                nc.sync.dma_start(out=dt_[:, :], in_=diag_ap)
                ta = actpool.tile([1, 512], F32, tag="diag_a")
                nc.scalar.activation(ta[:, :], dt_[:, :], AF.Abs)
                nc.scalar.activation(ta[:, :], ta[:, :], AF.Exp, scale=-1.0)
                nc.vector.tensor_scalar_add(out=ta[:, :], in0=ta[:, :], scalar1=1.0)
                nc.scalar.activation(ta[:, :], ta[:, :], AF.Ln)
                tr = actpool.tile([1, 512], F32, tag="diag_r")
                nc.scalar.activation(tr[:, :], dt_[:, :], AF.Relu)
                nc.vector.tensor_add(out=tr[:, :], in0=tr[:, :], in1=ta[:, :])
                nc.sync.dma_start(out=diag_ap, in_=tr[:, :])

    nc.compile()
    return nc


_NC_CACHE = None


def get_nc():
    global _NC_CACHE
    if _NC_CACHE is None:
        _NC_CACHE = build_unet()
    return _NC_CACHE


def make_in_maps(inputs):
    wmap = prep_weights(inputs)
    x = np.asarray(inputs["x"])  # [8, 512, 512, 1] f32
    in_maps = []
    for i in range(B):
        m = dict(wmap)
        m["x"] = np.ascontiguousarray(
            x[i, :, :, 0].reshape(-1).astype(STREAM_NP))
        in_maps.append(m)
    return in_maps


def kernel(_trace=False, **inputs):
    nc = get_nc()
    in_maps = make_in_maps(inputs)
    res = run_bass_kernel_spmd(nc, in_maps, core_ids=list(range(B)),
                               trace=_trace)
    out = np.stack([res.results[i]["out"] for i in range(B)], axis=0)
    out = out[:, :, :, None].astype(np.float32)
    if _trace:
        return out, res
    return out



# revision 13
# speedup vs baseline: 2.2586x; 2.1523x over previous
"""PreconditionerSparseUNet on 8 TRN2 NeuronCores (v2).

Sharding: data-parallel over batch (8 images, 1 per core); weights replicated.

v2 redesign vs v1 baseline:
- DRAM feature maps are [C, (h+2+GR)*W] fp16 with W=w+2: padded rows whose pad
  columns are written as part of contiguous store runs (no per-column zeroing
  descriptor storms; only tiny pad-row/guard zero fills at start).
- Output-row packing into matmul M: a psum tile computes s output rows x w cols
  for all cout channels (M = cout*s <= 128). K folds input channels x row-copy
  index; row copies are partition-views of per-tile row slots in SBUF.
- kx taps handled by <=3 accumulating matmuls at element-shifted rhs offsets.
- Stores write whole padded rows (pads pre-zeroed in SBUF act tiles at warmup),
  batched nb rows per DMA, >=512B runs for the wide maps.
- DMA issue spread across Sync / Vector / Pool(gpsimd) sequencers.
- Transposed convs keep the parity-class decomposition; skip-add fused before
  store.
"""

import numpy as np

import concourse.bass as bass
import concourse.bacc as bacc
import concourse.mybir as mybir
from concourse.tile import TileContext
from concourse.bass_utils import run_bass_kernel_spmd

AF = mybir.ActivationFunctionType
F32 = mybir.dt.float32
F16 = mybir.dt.float16

STREAM_DT = F16
STREAM_NP = np.float16

N = 512
B = 8
ALPHA = 0.01
GR = 12  # guard rows past the bottom pad row
NT = 12  # tiles per tin chunk (all layers)

# s1/s2 conv layers: name, C, Co, h_in, f(stride), s(dy-pack), wsrc, inb, outb
S12 = [
    ("enc1", 1, 16, 512, 1, 8, "w_enc1", "xp", "enc1p"),
    ("down1", 16, 32, 512, 2, 3, "w_down1", "enc1p", "down1p"),
    ("enc2", 32, 32, 256, 1, 4, "w_enc2", "down1p", "enc2p"),
    ("down2", 32, 64, 256, 2, 2, "w_down2", "enc2p", "down2p"),
    ("enc3", 64, 64, 128, 1, 2, "w_enc3", "down2p", "enc3p"),
    ("bn", 64, 128, 128, 2, 1, "w_bn", "enc3p", "bnp"),
    ("dec2", 64, 64, 128, 1, 2, "w_dec2", "up2p", "dec2p"),
    ("dec1", 32, 32, 256, 1, 4, "w_dec1", "up1p", "dec1p"),
    ("dec0", 16, 16, 512, 1, 6, "w_dec0", "up0p", "dec0p"),
]
TCONVS = [
    ("up2", 128, 64, 64, "w_up2", "bnp", "up2p", "enc3p"),
    ("up1", 64, 32, 128, "w_up1", "dec2p", "up1p", "enc2p"),
    ("up0", 32, 16, 256, "w_up0", "dec1p", "up0p", "enc1p"),
]
MAPS = {
    "xp": (1, 512), "enc1p": (16, 512), "down1p": (32, 256), "enc2p": (32, 256),
    "down2p": (64, 128), "enc3p": (64, 128), "bnp": (128, 64), "up2p": (64, 128),
    "dec2p": (64, 128), "up1p": (32, 256), "dec1p": (32, 256), "up0p": (16, 512),
    "dec0p": (16, 512),
}
ORDER = ["enc1", "down1", "enc2", "down2", "enc3", "bn",
         "up2", "dec2", "up1", "dec1", "up0", "dec0"]


def nj_of(f, s):
    return f * (s - 1) + 3


def jgroups_of(C, nj):
    g = max(1, 128 // C)
    return [list(range(a, min(a + g, nj))) for a in range(0, nj, g)]


def pack_stride(cout):
    return 32 if cout <= 32 else (64 if cout == 64 else 128)


def pmap(parity, d):
    if parity == 0:
        return 1 if d == 0 else None
    return 0 if d == 0 else 2


def tconv_plan(cin, w_in):
    W = w_in + 2
    if cin == 128:
        blocks = [0]
    elif cin == 64:
        blocks = [0, 1]
    elif cin == 32:
        blocks = [0, 1, W, W + 1]
    else:
        raise ValueError(cin)
    classes = []
    for py in range(2):
        for px in range(2):
            dis = [d for d in range(2) if pmap(py, d) is not None]
            djs = [d for d in range(2) if pmap(px, d) is not None]
            mms = []
            if cin == 128:
                for di in dis:
                    for dj in djs:
                        mms.append(dict(segs=[(pmap(py, di), pmap(px, dj))],
                                        r=di * W + dj))
            elif cin == 64:
                for di in dis:
                    if len(djs) == 2:
                        mms.append(dict(segs=[(pmap(py, di), pmap(px, 0)),
                                              (pmap(py, di), pmap(px, 1))],
                                        r=di * W))
                    else:
                        mms.append(dict(segs=[(pmap(py, di), 1)], r=di * W))
            else:  # cin == 32
                if py == 0 and px == 0:
                    mms = [dict(segs=[(1, 1)], r=0)]
                elif py == 0 and px == 1:
                    mms = [dict(segs=[(1, 0), (1, 2)], r=0)]
                elif py == 1 and px == 0:
                    mms = [dict(segs=[(0, 1), None, (2, 1), None], r=0)]
                else:
                    mms = [dict(segs=[(0, 0), (0, 2), (2, 0), (2, 2)], r=0)]
            classes.append((py, px, mms))
    return blocks, classes


def tconv_mm_keys(name, cin, w_in):
    out = []
    _, classes = tconv_plan(cin, w_in)
    for py, px, mms in classes:
        for i, m in enumerate(mms):
            out.append((f"W_{name}_c{py}{px}_{i}", m))
    return out


# ---------------------------------------------------------------------------
# Host-side input prep
# ---------------------------------------------------------------------------

def prep_weights(inputs):
    wmap = {}
    for (name, C, Co, h_in, f, s, wsrc, _i, _o) in S12:
        w = np.asarray(inputs[wsrc])  # [3,3,C,Co]
        M = Co * s
        if name == "enc1":
            lt = np.zeros((30, 128), np.float32)
            for j in range(10):
                for kx in range(3):
                    r = j * 3 + kx
                    for dy in range(8):
                        ky = j - dy
                        if 0 <= ky < 3:
                            lt[r, dy * 16:(dy + 1) * 16] = w[ky, kx, 0, :]
            wmap["W_enc1_0_0"] = np.ascontiguousarray(lt.astype(STREAM_NP))
        else:
            nj = nj_of(f, s)
            jgs = jgroups_of(C, nj)
            for kx in range(3):
                for gi, jg in enumerate(jgs):
                    lt = np.zeros((C * len(jg), M), np.float32)
                    for jj, j in enumerate(jg):
                        for dy in range(s):
                            ky = j - f * dy
                            if 0 <= ky < 3:
                                lt[jj * C:(jj + 1) * C,
                                   dy * Co:(dy + 1) * Co] = w[ky, kx]
                    wmap[f"W_{name}_{kx}_{gi}"] = np.ascontiguousarray(
                        lt.astype(STREAM_NP))
        b = np.asarray(inputs["b_" + wsrc[2:]]).astype(np.float32)
        b128 = np.zeros((128, 1), np.float32)
        b128[:M, 0] = np.tile(b, s)
        wmap[f"B_{name}"] = b128

    for (name, cin, cout, w_in, wsrc, *_r) in TCONVS:
        w = np.asarray(inputs[wsrc])
        for key, m in tconv_mm_keys(name, cin, w_in):
            segs = []
            for sg in m["segs"]:
                if sg is None:
                    segs.append(np.zeros((cin, cout), np.float32))
                else:
                    segs.append(w[sg[0], sg[1]])
            wmap[key] = np.ascontiguousarray(
                np.concatenate(segs, axis=0).astype(STREAM_NP))
        b = np.asarray(inputs["b_" + wsrc[2:]]).astype(np.float32)
        stride = pack_stride(cout)
        b128 = np.zeros((128, 1), np.float32)
        for pk in range(128 // stride):
            b128[pk * stride: pk * stride + cout, 0] = b
        wmap[f"B_{name}"] = b128

    wo = np.asarray(inputs["w_out"]).reshape(16)
    lt = np.zeros((128, 8), np.float32)
    for j in range(8):
        lt[j * 16:(j + 1) * 16, j] = wo
    wmap["W_out"] = np.ascontiguousarray(lt.astype(STREAM_NP))
    wmap["B_out"] = np.full((128, 1), float(np.asarray(inputs["b_out"])[0]),
                            np.float32)

    # masku [128, 16*512] f32: bank T covers rows T*32 + q*8 + d
    tri = np.tril(np.ones((N, N), np.float32))
    mk = np.zeros((128, 16 * 512), np.float32)
    for p in range(128):
        q, d = divmod(p, 32)
        if d < 8:
            for tb in range(16):
                mk[p, tb * 512:(tb + 1) * 512] = tri[tb * 32 + q * 8 + d]
    wmap["MASKU"] = mk.astype(STREAM_NP)
    return wmap


def make_in_maps(inputs):
    wmap = prep_weights(inputs)
    x = np.asarray(inputs["x"])  # [8,512,512,1]
    W0 = N + 2
    in_maps = []
    for i in range(B):
        m = dict(wmap)
        xp = np.zeros((N + 2 + GR, W0), np.float32)
        xp[1:N + 1, 1:N + 1] = x[i, :, :, 0]
        m["xp"] = np.ascontiguousarray(xp.reshape(1, -1).astype(STREAM_NP))
        in_maps.append(m)
    return in_maps


# ---------------------------------------------------------------------------
# Kernel builder
# ---------------------------------------------------------------------------

def ap_of(base_ap, off, dims):
    return bass.AP(base_ap.tensor, base_ap.offset + off,
                   [list(d) for d in dims])


def build_unet():
    nc = bacc.Bacc("TRN2", target_bir_lowering=False, debug=False)

    bufs = {}
    for nm, (C, h) in MAPS.items():
        W = h + 2
        kind = "ExternalInput" if nm == "xp" else "Internal"
        bufs[nm] = nc.dram_tensor(nm, [C, (h + 2 + GR) * W], STREAM_DT,
                                  kind=kind).ap()
    out_t = nc.dram_tensor("out", [N, N], F32, kind="ExternalOutput").ap()

    win = {}

    def declare(key, shape, dt):
        win[key] = nc.dram_tensor(key, shape, dt, kind="ExternalInput").ap()

    for (name, C, Co, h_in, f, s, wsrc, _i, _o) in S12:
        if name == "enc1":
            declare("W_enc1_0_0", [30, 128], STREAM_DT)
        else:
            nj = nj_of(f, s)
            for kx in range(3):
                for gi, jg in enumerate(jgroups_of(C, nj)):
                    declare(f"W_{name}_{kx}_{gi}", [C * len(jg), Co * s],
                            STREAM_DT)
        declare(f"B_{name}", [128, 1], F32)
    for (name, cin, cout, w_in, *_r) in TCONVS:
        for key, m in tconv_mm_keys(name, cin, w_in):
            k = len(m["segs"]) * cin
            declare(key, [k, cout], STREAM_DT)
        declare(f"B_{name}", [128, 1], F32)
    declare("W_out", [128, 8], STREAM_DT)
    declare("B_out", [128, 1], F32)
    declare("MASKU", [128, 16 * 512], STREAM_DT)

    with TileContext(nc) as tc:
        with (
            tc.tile_pool(name="wpool", bufs=1) as wpool,
            tc.tile_pool(name="inpool", bufs=2) as inpool,
            tc.tile_pool(name="actpool", bufs=2) as actpool,
            tc.tile_pool(name="skpool", bufs=2) as skpool,
            tc.tile_pool(name="psum", bufs=8, space="PSUM") as pspool,
        ):
            wt = {}
            for key, ap in win.items():
                kdim, mdim = ap.ap[0][1], ap.ap[1][1]
                dt = F32 if key.startswith("B_") else STREAM_DT
                t = wpool.tile([128, mdim], dt, tag=key)
                nc.sync.dma_start(out=t[0:kdim, :], in_=ap)
                wt[key] = t

            ZW = 4 * 514
            zt = wpool.tile([128, ZW], STREAM_DT, tag="zeros")
            nc.vector.memset(zt[:, :], 0.0)

            # zero pad rows + guard rows of all internal maps
            for nm, (C, h) in MAPS.items():
                if nm == "xp":
                    continue
                W = h + 2
                Lp = (h + 2 + GR) * W
                bap = bufs[nm]
                nc.sync.dma_start(
                    out=ap_of(bap, 0, [[Lp, C], [1, W]]),
                    in_=zt[0:C, 0:W])
                g = (1 + GR) * W
                o = 0
                while o < g:
                    c = min(ZW, g - o)
                    nc.sync.dma_start(
                        out=ap_of(bap, (h + 1) * W + o, [[Lp, C], [1, c]]),
                        in_=zt[0:C, 0:c])
                    o += c

            # ---- s1/s2 emitter -------------------------------------------
            def emit_s12(li, name, C, Co, h_in, f, s, inb, outb):
                h_out = h_in // f
                w_out = h_out
                Wi, Wo = h_in + 2, w_out + 2
                Lpi = (h_in + 2 + GR) * Wi
                Lpo = (h_out + 2 + GR) * Wo
                M = Co * s
                nb = max(1, 512 // w_out)
                Nn = w_out
                T_out = (h_out + s - 1) // s
                bias = wt[f"B_{name}"][:, :]
                SG = 4 if w_out == 512 else 8   # tiles per store DMA
                inap = bufs[inb]
                outap = bufs[outb]

                if name == "enc1":
                    mms = [("W_enc1_0_0", None, 30, 0)]
                else:
                    njn = nj_of(f, s)
                    jgs = jgroups_of(C, njn)
                    mms = []
                    for kx in range(3):
                        for gi, jg in enumerate(jgs):
                            mms.append((f"W_{name}_{kx}_{gi}", gi,
                                        C * len(jg), kx))

                atag = f"act_{name}"
                for _ in range(2):
                    t = actpool.tile([128, SG * Wo], STREAM_DT, tag=atag)
                    nc.vector.memset(t[:, :], 0.0)

                t0 = 0
                while t0 < T_out:
                    nt = min(NT, T_out - t0)
                    span = nt * Wi
                    tins = {}
                    if name == "enc1":
                        ti = inpool.tile([30, NT * Wi], STREAM_DT, tag="tinA")
                        nc.sync.dma_start(
                            out=ti[0:30, 0:span],
                            in_=ap_of(inap, (8 * t0) * Wi,
                                      [[Wi, 10], [1, 3], [8 * Wi, nt],
                                       [1, Wi]]))
                        tins[0] = ti
                    else:
                        for gi, jg in enumerate(jgs):
                            gj = len(jg)
                            ti = inpool.tile([C * gj, NT * Wi], STREAM_DT,
                                             tag=("tinA" if gi == 0 else "tinB"))
                            nc.sync.dma_start(
                                out=ti[0:C * gj, 0:span],
                                in_=ap_of(inap, (f * s * t0 + jg[0]) * Wi,
                                          [[Wi, gj], [Lpi, C],
                                           [f * s * Wi, nt], [1, Wi]]))
                            tins[gi] = ti

                    sg0 = t0
                    while sg0 < t0 + nt:
                        sgn = min(SG, t0 + nt - sg0)
                        act = actpool.tile([128, SG * Wo], STREAM_DT, tag=atag)
                        aap = act[:, :]
                        apitch = aap.ap[0][0]
                        tb = sg0
                        while tb < sg0 + sgn:
                            qn = min(nb, sg0 + sgn - tb)
                            ps = pspool.tile([128, 512], F32, tag="ps")
                            for q in range(qn):
                                t = tb + q
                                for mi, (key, gi, K, kx) in enumerate(mms):
                                    tin = tins[0] if gi is None else tins[gi]
                                    tap = tin[:, :]
                                    roff = (t - t0) * Wi + (0 if gi is None else kx)
                                    rhs = bass.AP(tap.tensor, tap.offset + roff,
                                                  [[tap.ap[0][0], K], [f, w_out]])
                                    nc.tensor.matmul(
                                        ps[0:M, q * Nn:(q + 1) * Nn],
                                        lhsT=wt[key][0:K, 0:M], rhs=rhs,
                                        start=(mi == 0), stop=(mi == len(mms) - 1),
                                        tile_position=(0, 0))
                            oact = bass.AP(aap.tensor,
                                           aap.offset + (tb - sg0) * Wo + 1,
                                           [[apitch, 128], [Wo, qn], [1, w_out]])
                            pap = ps[:, :]
                            ips = bass.AP(pap.tensor, pap.offset,
                                          [[pap.ap[0][0], 128], [Nn, qn],
                                           [1, w_out]])
                            nc.scalar.activation(oact, ips, AF.Prelu,
                                                 bias=bias, alpha=ALPHA)
                            tb += qn
                        nc.sync.dma_start(
                            out=ap_of(outap, (1 + sg0 * s) * Wo,
                                      [[Lpo, Co], [Wo, s], [s * Wo, sgn],
                                       [1, Wo]]),
                            in_=bass.AP(aap.tensor, aap.offset,
                                        [[apitch, M], [1, sgn * Wo]]))
                        sg0 += sgn
                    t0 += nt

                if T_out * s != h_out:
                    junk = T_out * s - h_out + 1
                    g = (junk + 1) * Wo
                    o = 0
                    while o < g:
                        c = min(2048, g - o)
                        nc.sync.dma_start(
                            out=ap_of(outap, (h_out + 1) * Wo + o,
                                      [[Lpo, Co], [1, c]]),
                            in_=zt[0:Co, 0:c])
                        o += c

            # ---- tconv emitter -------------------------------------------
            def emit_tconv(li, name, cin, cout, w_in, inb, outb, skipb):
                Wi = w_in + 2
                w_out = 2 * w_in
                Wo = w_out + 2
                Lpi = (w_in + 2 + GR) * Wi
                Lpo = (w_out + 2 + GR) * Wo
                blocks, classes = tconv_plan(cin, w_in)
                cls = {(py, px): mms for (py, px, mms) in classes}
                stride = pack_stride(cout)
                pack = 128 // stride
                Ri = 512 // w_in
                rows_pt = pack * Ri
                bias = wt[f"B_{name}"][:, :]
                Rg = {64: 64, 128: 32, 256: 16}[w_in]
                inap, outap, skap = bufs[inb], bufs[outb], bufs[skipb]

                wtag = f"wide_{name}"
                for _ in range(2):
                    t = actpool.tile([128, Ri * Wo], STREAM_DT, tag=wtag)
                    nc.vector.memset(t[:, :], 0.0)

                for i0 in range(0, w_in, Rg):
                    A0 = (1 + i0) * Wi + 1
                    span = (Rg + 2) * Wi
                    tin = inpool.tile([len(blocks) * cin, (Rg + 2) * Wi],
                                      STREAM_DT, tag="tin_tc")
                    for j, sft in enumerate(blocks):
                        nc.sync.dma_start(
                            out=tin[j * cin:(j + 1) * cin, 0:span],
                            in_=ap_of(inap, A0 + sft,
                                      [[Lpi, cin], [1, span]]))
                    tin_ap = tin[:, :]
                    tpitch = tin_ap.ap[0][0]
                    for py in range(2):
                        for it in range(0, Rg, rows_pt):
                            ps_pair = []
                            for px in range(2):
                                mms = cls[(py, px)]
                                ps = pspool.tile([128, 512], F32, tag="ps")
                                for pk in range(pack):
                                    col = pk * stride
                                    base = (it + pk * Ri) * Wi
                                    for mi, m in enumerate(mms):
                                        K = len(m["segs"]) * cin
                                        rhs = bass.AP(
                                            tin_ap.tensor,
                                            tin_ap.offset + base + m["r"],
                                            [[tpitch, K], [Wi, Ri], [1, w_in]])
                                        nc.tensor.matmul(
                                            ps[col:col + cout, 0:Ri * w_in],
                                            lhsT=wt[f"W_{name}_c{py}{px}_{mi}"][0:K, 0:cout],
                                            rhs=rhs,
                                            start=(mi == 0),
                                            stop=(mi == len(mms) - 1),
                                            tile_position=(0, col))
                                ps_pair.append(ps)
                            wide = actpool.tile([128, Ri * Wo], STREAM_DT,
                                                tag=wtag)
                            wap = wide[:, :]
                            wpitch = wap.ap[0][0]
                            for px in range(2):
                                pap = ps_pair[px][:, :]
                                oap = bass.AP(wap.tensor,
                                              wap.offset + 1 + px,
                                              [[wpitch, 128], [Wo, Ri],
                                               [2, w_in]])
                                iap = bass.AP(pap.tensor, pap.offset,
                                              [[pap.ap[0][0], 128],
                                               [w_in, Ri], [1, w_in]])
                                nc.scalar.activation(oap, iap, AF.Prelu,
                                                     bias=bias, alpha=ALPHA)
                            skt = skpool.tile([128, Ri * Wo], STREAM_DT,
                                              tag="skt")
                            sap = skt[:, :]
                            spitch = sap.ap[0][0]
                            io0 = i0 + it
                            off0 = (1 + 2 * io0 + py) * Wo
                            nc.sync.dma_start(
                                out=bass.AP(sap.tensor, sap.offset,
                                            [[stride * spitch, pack],
                                             [spitch, cout], [Wo, Ri],
                                             [1, Wo]]),
                                in_=ap_of(skap, off0,
                                          [[2 * Ri * Wo, pack], [Lpo, cout],
                                           [2 * Wo, Ri], [1, Wo]]))
                            nc.vector.tensor_add(out=wap, in0=wap, in1=sap)
                            nc.sync.dma_start(
                                out=ap_of(outap, off0,
                                          [[2 * Ri * Wo, pack], [Lpo, cout],
                                           [2 * Wo, Ri], [1, Wo]]),
                                in_=bass.AP(wap.tensor, wap.offset,
                                            [[stride * wpitch, pack],
                                             [wpitch, cout], [Wo, Ri],
                                             [1, Wo]]))

            s12map = {r[0]: r for r in S12}
            tcmap = {r[0]: r for r in TCONVS}
            for li, nm in enumerate(ORDER):
                if nm in s12map:
                    (name, C, Co, h_in, f, s, wsrc, inb, outb) = s12map[nm]
                    emit_s12(li, name, C, Co, h_in, f, s, inb, outb)
                else:
                    (name, cin, cout, w_in, wsrc, inb, outb, skipb) = tcmap[nm]
                    emit_tconv(li, name, cin, cout, w_in, inb, outb, skipb)

            # ---- out layer: 1x1 conv + tri mask --------------------------
            W0 = 514
            Lp0 = (N + 2 + GR) * W0
            d0 = bufs["dec0p"]
            masku = wt["MASKU"][:, :]
            mpitch = masku.ap[0][0]
            bout = wt["B_out"][:, :]
            for ci, t0 in enumerate(range(0, 64, NT)):
                nt = min(NT, 64 - t0)
                span = nt * W0
                tin = inpool.tile([128, NT * W0], STREAM_DT, tag="tinA")
                nc.sync.dma_start(
                    out=tin[0:128, 0:span],
                    in_=ap_of(d0, (1 + 8 * t0) * W0,
                              [[W0, 8], [Lp0, 16], [8 * W0, nt], [1, W0]]))
                tap = tin[:, :]
                tpitch = tap.ap[0][0]
                for bb in range(nt // 4):   # banks of 4 tiles (32 rows)
                    ps = pspool.tile([128, 512], F32, tag="ps")
                    for q in range(4):
                        rhs = bass.AP(tap.tensor,
                                      tap.offset + (bb * 4 + q) * W0 + 1,
                                      [[tpitch, 128], [1, 512]])
                        nc.tensor.matmul(ps[q * 32:q * 32 + 8, 0:512],
                                         lhsT=wt["W_out"][0:128, 0:8], rhs=rhs,
                                         start=True, stop=True,
                                         tile_position=(0, q * 32))
                    a16 = actpool.tile([128, 512], STREAM_DT, tag="a16_out")
                    nc.scalar.activation(a16[:, :], ps[:, :], AF.Identity,
                                         bias=bout)
                    T = (t0 + bb * 4) // 4   # bank index 0..15
                    mslice = bass.AP(masku.tensor, masku.offset + T * 512,
                                     [[mpitch, 128], [1, 512]])
                    nc.vector.tensor_mul(out=a16[:, :], in0=a16[:, :],
                                         in1=mslice)
                    act = actpool.tile([128, 512], F32, tag="act_out")
                    nc.vector.tensor_scalar_add(out=act[:, :], in0=a16[:, :],
                                                scalar1=0.0)
                    for q in range(4):
                        nc.sync.dma_start(
                            out=ap_of(out_t, (T * 32 + q * 8) * 512,
                                      [[512, 8], [1, 512]]),
                            in_=act[q * 32:q * 32 + 8, 0:512])

            # ---- diagonal softplus patch ---------------------------------
            out_flat = out_t.flatten()
            diag_ap = bass.AP(out_flat.tensor, out_flat.offset, [[513, 512]])
            dt_ = actpool.tile([1, 512], F32, tag="diag")
            nc.sync.dma_start(out=dt_[:, :], in_=diag_ap)
            ta = actpool.tile([1, 512], F32, tag="diag_a")
            nc.scalar.activation(ta[:, :], dt_[:, :], AF.Abs)
            nc.scalar.activation(ta[:, :], ta[:, :], AF.Exp, scale=-1.0)
            nc.vector.tensor_scalar_add(out=ta[:, :], in0=ta[:, :], scalar1=1.0)
            nc.scalar.activation(ta[:, :], ta[:, :], AF.Ln)
            tr = actpool.tile([1, 512], F32, tag="diag_r")
            nc.scalar.activation(tr[:, :], dt_[:, :], AF.Relu)
            nc.vector.tensor_add(out=tr[:, :], in0=tr[:, :], in1=ta[:, :])
            nc.sync.dma_start(out=diag_ap, in_=tr[:, :])

    nc.compile()
    return nc


_NC_CACHE = None


def get_nc():
    global _NC_CACHE
    if _NC_CACHE is None:
        _NC_CACHE = build_unet()
    return _NC_CACHE


def kernel(_trace=False, **inputs):
    nc = get_nc()
    in_maps = make_in_maps(inputs)
    res = run_bass_kernel_spmd(nc, in_maps, core_ids=list(range(B)),
                               trace=_trace)
    out = np.stack([res.results[i]["out"] for i in range(B)], axis=0)
    out = out[:, :, :, None].astype(np.float32)
    if _trace:
        return out, res
    return out


# revision 14
# speedup vs baseline: 3.1495x; 1.3945x over previous
"""PreconditionerSparseUNet on 8 TRN2 NeuronCores (v2).

Sharding: data-parallel over batch (8 images, 1 per core); weights replicated.

v2 redesign vs v1 baseline:
- DRAM feature maps are [C, (h+2+GR)*W] fp16 with W=w+2: padded rows whose pad
  columns are written as part of contiguous store runs (no per-column zeroing
  descriptor storms; only tiny pad-row/guard zero fills at start).
- Output-row packing into matmul M: a psum tile computes s output rows x w cols
  for all cout channels (M = cout*s <= 128). K folds input channels x row-copy
  index; row copies are partition-views of per-tile row slots in SBUF.
- kx taps handled by <=3 accumulating matmuls at element-shifted rhs offsets.
- Stores write whole padded rows (pads pre-zeroed in SBUF act tiles at warmup),
  batched nb rows per DMA, >=512B runs for the wide maps.
- DMA issue spread across Sync / Vector / Pool(gpsimd) sequencers.
- Transposed convs keep the parity-class decomposition; skip-add fused before
  store.
"""

import numpy as np

import concourse.bass as bass
import concourse.bacc as bacc
import concourse.mybir as mybir
from concourse.tile import TileContext
from concourse.bass_utils import run_bass_kernel_spmd

AF = mybir.ActivationFunctionType
F32 = mybir.dt.float32
F16 = mybir.dt.float16

STREAM_DT = F16
STREAM_NP = np.float16

N = 512
B = 8
ALPHA = 0.01
GR = 12  # guard rows past the bottom pad row
NT = 12  # tiles per tin chunk (all layers)

# s1/s2 conv layers: name, C, Co, h_in, f(stride), s(dy-pack), wsrc, inb, outb
S12 = [
    ("enc1", 1, 16, 512, 1, 8, "w_enc1", "xp", "enc1p"),
    ("down1", 16, 32, 512, 2, 3, "w_down1", "enc1p", "down1p"),
    ("enc2", 32, 32, 256, 1, 4, "w_enc2", "down1p", "enc2p"),
    ("down2", 32, 64, 256, 2, 2, "w_down2", "enc2p", "down2p"),
    ("enc3", 64, 64, 128, 1, 2, "w_enc3", "down2p", "enc3p"),
    ("bn", 64, 128, 128, 2, 1, "w_bn", "enc3p", "bnp"),
    ("dec2", 64, 64, 128, 1, 2, "w_dec2", "up2p", "dec2p"),
    ("dec1", 32, 32, 256, 1, 4, "w_dec1", "up1p", "dec1p"),
    ("dec0", 16, 16, 512, 1, 6, "w_dec0", "up0p", "dec0p"),
]
TCONVS = [
    ("up2", 128, 64, 64, "w_up2", "bnp", "up2p", "enc3p"),
    ("up1", 64, 32, 128, "w_up1", "dec2p", "up1p", "enc2p"),
    ("up0", 32, 16, 256, "w_up0", "dec1p", "up0p", "enc1p"),
]
MAPS = {
    "xp": (1, 512), "enc1p": (16, 512), "down1p": (32, 256), "enc2p": (32, 256),
    "down2p": (64, 128), "enc3p": (64, 128), "bnp": (128, 64), "up2p": (64, 128),
    "dec2p": (64, 128), "up1p": (32, 256), "dec1p": (32, 256), "up0p": (16, 512),
    "dec0p": (16, 512),
}
ORDER = ["enc1", "down1", "enc2", "down2", "enc3", "bn",
         "up2", "dec2", "up1", "dec1", "up0", "dec0"]


def nj_of(f, s):
    return f * (s - 1) + 3


def jgroups_of(C, nj):
    g = max(1, 128 // C)
    return [list(range(a, min(a + g, nj))) for a in range(0, nj, g)]


def pack_stride(cout):
    return 32 if cout <= 32 else (64 if cout == 64 else 128)


def pmap(parity, d):
    if parity == 0:
        return 1 if d == 0 else None
    return 0 if d == 0 else 2


def tconv_plan(cin, w_in):
    W = w_in + 2
    if cin == 128:
        blocks = [0]
    elif cin == 64:
        blocks = [0, 1]
    elif cin == 32:
        blocks = [0, 1, W, W + 1]
    else:
        raise ValueError(cin)
    classes = []
    for py in range(2):
        for px in range(2):
            dis = [d for d in range(2) if pmap(py, d) is not None]
            djs = [d for d in range(2) if pmap(px, d) is not None]
            mms = []
            if cin == 128:
                for di in dis:
                    for dj in djs:
                        mms.append(dict(segs=[(pmap(py, di), pmap(px, dj))],
                                        r=di * W + dj))
            elif cin == 64:
                for di in dis:
                    if len(djs) == 2:
                        mms.append(dict(segs=[(pmap(py, di), pmap(px, 0)),
                                              (pmap(py, di), pmap(px, 1))],
                                        r=di * W))
                    else:
                        mms.append(dict(segs=[(pmap(py, di), 1)], r=di * W))
            else:  # cin == 32
                if py == 0 and px == 0:
                    mms = [dict(segs=[(1, 1)], r=0)]
                elif py == 0 and px == 1:
                    mms = [dict(segs=[(1, 0), (1, 2)], r=0)]
                elif py == 1 and px == 0:
                    mms = [dict(segs=[(0, 1), None, (2, 1), None], r=0)]
                else:
                    mms = [dict(segs=[(0, 0), (0, 2), (2, 0), (2, 2)], r=0)]
            classes.append((py, px, mms))
    return blocks, classes


def tconv_mm_keys(name, cin, w_in):
    out = []
    _, classes = tconv_plan(cin, w_in)
    for py, px, mms in classes:
        for i, m in enumerate(mms):
            out.append((f"W_{name}_c{py}{px}_{i}", m))
    return out


# ---------------------------------------------------------------------------
# Host-side input prep
# ---------------------------------------------------------------------------

def prep_weights(inputs):
    wmap = {}
    for (name, C, Co, h_in, f, s, wsrc, _i, _o) in S12:
        w = np.asarray(inputs[wsrc])  # [3,3,C,Co]
        M = Co * s
        if name == "enc1":
            lt = np.zeros((30, 128), np.float32)
            for j in range(10):
                for kx in range(3):
                    r = j * 3 + kx
                    for dy in range(8):
                        ky = j - dy
                        if 0 <= ky < 3:
                            lt[r, dy * 16:(dy + 1) * 16] = w[ky, kx, 0, :]
            wmap["W_enc1_0_0"] = np.ascontiguousarray(lt.astype(STREAM_NP))
        else:
            nj = nj_of(f, s)
            jgs = jgroups_of(C, nj)
            for kx in range(3):
                for gi, jg in enumerate(jgs):
                    lt = np.zeros((C * len(jg), M), np.float32)
                    for jj, j in enumerate(jg):
                        for dy in range(s):
                            ky = j - f * dy
                            if 0 <= ky < 3:
                                lt[jj * C:(jj + 1) * C,
                                   dy * Co:(dy + 1) * Co] = w[ky, kx]
                    wmap[f"W_{name}_{kx}_{gi}"] = np.ascontiguousarray(
                        lt.astype(STREAM_NP))
        b = np.asarray(inputs["b_" + wsrc[2:]]).astype(np.float32)
        b128 = np.zeros((128, 1), np.float32)
        b128[:M, 0] = np.tile(b, s)
        wmap[f"B_{name}"] = b128

    for (name, cin, cout, w_in, wsrc, *_r) in TCONVS:
        w = np.asarray(inputs[wsrc])
        for key, m in tconv_mm_keys(name, cin, w_in):
            segs = []
            for sg in m["segs"]:
                if sg is None:
                    segs.append(np.zeros((cin, cout), np.float32))
                else:
                    segs.append(w[sg[0], sg[1]])
            wmap[key] = np.ascontiguousarray(
                np.concatenate(segs, axis=0).astype(STREAM_NP))
        b = np.asarray(inputs["b_" + wsrc[2:]]).astype(np.float32)
        stride = pack_stride(cout)
        b128 = np.zeros((128, 1), np.float32)
        for pk in range(128 // stride):
            b128[pk * stride: pk * stride + cout, 0] = b
        wmap[f"B_{name}"] = b128

    wo = np.asarray(inputs["w_out"]).reshape(16)
    lt = np.zeros((128, 8), np.float32)
    for j in range(8):
        lt[j * 16:(j + 1) * 16, j] = wo
    wmap["W_out"] = np.ascontiguousarray(lt.astype(STREAM_NP))
    wmap["B_out"] = np.full((128, 1), float(np.asarray(inputs["b_out"])[0]),
                            np.float32)

    # masku [128, 16*512] f32: bank T covers rows T*32 + q*8 + d
    tri = np.tril(np.ones((N, N), np.float32))
    mk = np.zeros((128, 16 * 512), np.float32)
    for p in range(128):
        q, d = divmod(p, 32)
        if d < 8:
            for tb in range(16):
                mk[p, tb * 512:(tb + 1) * 512] = tri[tb * 32 + q * 8 + d]
    wmap["MASKU"] = mk.astype(STREAM_NP)
    return wmap


def make_in_maps(inputs):
    wmap = prep_weights(inputs)
    x = np.asarray(inputs["x"])  # [8,512,512,1]
    W0 = N + 2
    in_maps = []
    for i in range(B):
        m = dict(wmap)
        xp = np.zeros((N + 2 + GR, W0), np.float32)
        xp[1:N + 1, 1:N + 1] = x[i, :, :, 0]
        m["xp"] = np.ascontiguousarray(xp.reshape(1, -1).astype(STREAM_NP))
        in_maps.append(m)
    return in_maps


# ---------------------------------------------------------------------------
# Kernel builder
# ---------------------------------------------------------------------------

def ap_of(base_ap, off, dims):
    return bass.AP(base_ap.tensor, base_ap.offset + off,
                   [list(d) for d in dims])


def build_unet():
    nc = bacc.Bacc("TRN2", target_bir_lowering=False, debug=False)

    bufs = {}
    for nm, (C, h) in MAPS.items():
        W = h + 2
        kind = "ExternalInput" if nm == "xp" else "Internal"
        bufs[nm] = nc.dram_tensor(nm, [C, (h + 2 + GR) * W], STREAM_DT,
                                  kind=kind).ap()
    out_t = nc.dram_tensor("out", [N, N], F32, kind="ExternalOutput").ap()

    win = {}

    def declare(key, shape, dt):
        win[key] = nc.dram_tensor(key, shape, dt, kind="ExternalInput").ap()

    for (name, C, Co, h_in, f, s, wsrc, _i, _o) in S12:
        if name == "enc1":
            declare("W_enc1_0_0", [30, 128], STREAM_DT)
        else:
            nj = nj_of(f, s)
            for kx in range(3):
                for gi, jg in enumerate(jgroups_of(C, nj)):
                    declare(f"W_{name}_{kx}_{gi}", [C * len(jg), Co * s],
                            STREAM_DT)
        declare(f"B_{name}", [128, 1], F32)
    for (name, cin, cout, w_in, *_r) in TCONVS:
        for key, m in tconv_mm_keys(name, cin, w_in):
            k = len(m["segs"]) * cin
            declare(key, [k, cout], STREAM_DT)
        declare(f"B_{name}", [128, 1], F32)
    declare("W_out", [128, 8], STREAM_DT)
    declare("B_out", [128, 1], F32)
    declare("MASKU", [128, 16 * 512], STREAM_DT)

    with TileContext(nc) as tc:
        with (
            tc.tile_pool(name="wpool", bufs=1) as wpool,
            tc.tile_pool(name="inpool", bufs=2) as inpool,
            tc.tile_pool(name="actpool", bufs=2) as actpool,
            tc.tile_pool(name="skpool", bufs=2) as skpool,
            tc.tile_pool(name="psum", bufs=8, space="PSUM") as pspool,
        ):
            wt = {}
            for key, ap in win.items():
                kdim, mdim = ap.ap[0][1], ap.ap[1][1]
                dt = F32 if key.startswith("B_") else STREAM_DT
                t = wpool.tile([128, mdim], dt, tag=key)
                nc.sync.dma_start(out=t[0:kdim, :], in_=ap)
                wt[key] = t

            ZW = 4 * 514
            zt = wpool.tile([128, ZW], STREAM_DT, tag="zeros")
            nc.vector.memset(zt[:, :], 0.0)

            # zero pad rows + guard rows of all internal maps
            for nm, (C, h) in MAPS.items():
                if nm == "xp":
                    continue
                W = h + 2
                Lp = (h + 2 + GR) * W
                bap = bufs[nm]
                nc.sync.dma_start(
                    out=ap_of(bap, 0, [[Lp, C], [1, W]]),
                    in_=zt[0:C, 0:W])
                g = (1 + GR) * W
                o = 0
                while o < g:
                    c = min(ZW, g - o)
                    nc.sync.dma_start(
                        out=ap_of(bap, (h + 1) * W + o, [[Lp, C], [1, c]]),
                        in_=zt[0:C, 0:c])
                    o += c

            # ---- s1/s2 emitter -------------------------------------------
            def emit_s12(li, name, C, Co, h_in, f, s, inb, outb):
                h_out = h_in // f
                w_out = h_out
                Wi, Wo = h_in + 2, w_out + 2
                Lpi = (h_in + 2 + GR) * Wi
                Lpo = (h_out + 2 + GR) * Wo
                M = Co * s
                nb = max(1, 512 // w_out)
                Nn = w_out
                T_out = (h_out + s - 1) // s
                bias = wt[f"B_{name}"][:, :]
                SG = 4 if w_out == 512 else 8   # tiles per store DMA
                inap = bufs[inb]
                outap = bufs[outb]

                if name == "enc1":
                    mms = [("W_enc1_0_0", None, 30, 0)]
                else:
                    njn = nj_of(f, s)
                    jgs = jgroups_of(C, njn)
                    mms = []
                    for kx in range(3):
                        for gi, jg in enumerate(jgs):
                            mms.append((f"W_{name}_{kx}_{gi}", gi,
                                        C * len(jg), kx))

                atag = f"act_{name}"
                for _ in range(2):
                    t = actpool.tile([128, SG * Wo], STREAM_DT, tag=atag)
                    nc.vector.memset(t[:, :], 0.0)

                t0 = 0
                while t0 < T_out:
                    nt = min(NT, T_out - t0)
                    span = nt * Wi
                    tins = {}
                    if name == "enc1":
                        ti = inpool.tile([30, NT * Wi], STREAM_DT, tag="tinA")
                        nc.sync.dma_start(
                            out=ti[0:30, 0:span],
                            in_=ap_of(inap, (8 * t0) * Wi,
                                      [[Wi, 10], [1, 3], [8 * Wi, nt],
                                       [1, Wi]]))
                        tins[0] = ti
                    else:
                        for gi, jg in enumerate(jgs):
                            gj = len(jg)
                            ti = inpool.tile([C * gj, NT * Wi], STREAM_DT,
                                             tag=("tinA" if gi == 0 else "tinB"))
                            nc.sync.dma_start(
                                out=ti[0:C * gj, 0:span],
                                in_=ap_of(inap, (f * s * t0 + jg[0]) * Wi,
                                          [[Wi, gj], [Lpi, C],
                                           [f * s * Wi, nt], [1, Wi]]))
                            tins[gi] = ti

                    sg0 = t0
                    while sg0 < t0 + nt:
                        sgn = min(SG, t0 + nt - sg0)
                        act = actpool.tile([128, SG * Wo], STREAM_DT, tag=atag)
                        aap = act[:, :]
                        apitch = aap.ap[0][0]
                        tb = sg0
                        while tb < sg0 + sgn:
                            qn = min(nb, sg0 + sgn - tb)
                            ps = pspool.tile([128, 512], F32, tag="ps")
                            for q in range(qn):
                                t = tb + q
                                for mi, (key, gi, K, kx) in enumerate(mms):
                                    tin = tins[0] if gi is None else tins[gi]
                                    tap = tin[:, :]
                                    roff = (t - t0) * Wi + (0 if gi is None else kx)
                                    rhs = bass.AP(tap.tensor, tap.offset + roff,
                                                  [[tap.ap[0][0], K], [f, w_out]])
                                    nc.tensor.matmul(
                                        ps[0:M, q * Nn:(q + 1) * Nn],
                                        lhsT=wt[key][0:K, 0:M], rhs=rhs,
                                        start=(mi == 0), stop=(mi == len(mms) - 1),
                                        tile_position=(0, 0))
                            oact = bass.AP(aap.tensor,
                                           aap.offset + (tb - sg0) * Wo + 1,
                                           [[apitch, 128], [Wo, qn], [1, w_out]])
                            pap = ps[:, :]
                            ips = bass.AP(pap.tensor, pap.offset,
                                          [[pap.ap[0][0], 128], [Nn, qn],
                                           [1, w_out]])
                            nc.scalar.activation(oact, ips, AF.Prelu,
                                                 bias=bias, alpha=ALPHA)
                            tb += qn
                        nc.sync.dma_start(
                            out=ap_of(outap, (1 + sg0 * s) * Wo,
                                      [[Lpo, Co], [Wo, s], [s * Wo, sgn],
                                       [1, Wo]]),
                            in_=bass.AP(aap.tensor, aap.offset,
                                        [[apitch, M], [1, sgn * Wo]]))
                        sg0 += sgn
                    t0 += nt

                if T_out * s != h_out:
                    junk = T_out * s - h_out + 1
                    g = (junk + 1) * Wo
                    o = 0
                    while o < g:
                        c = min(2048, g - o)
                        nc.sync.dma_start(
                            out=ap_of(outap, (h_out + 1) * Wo + o,
                                      [[Lpo, Co], [1, c]]),
                            in_=zt[0:Co, 0:c])
                        o += c

            # ---- tconv emitter -------------------------------------------
            def emit_tconv(li, name, cin, cout, w_in, inb, outb, skipb):
                Wi = w_in + 2
                w_out = 2 * w_in
                Wo = w_out + 2
                Lpi = (w_in + 2 + GR) * Wi
                Lpo = (w_out + 2 + GR) * Wo
                blocks, classes = tconv_plan(cin, w_in)
                cls = {(py, px): mms for (py, px, mms) in classes}
                stride = pack_stride(cout)
                pack = 128 // stride
                Ri = 512 // w_in
                rows_pt = pack * Ri
                bias = wt[f"B_{name}"][:, :]
                Rg = {64: 64, 128: 32, 256: 16}[w_in]
                skeng = nc.gpsimd if li % 2 == 0 else nc.sync
                steng = nc.sync if li % 2 == 0 else nc.gpsimd
                inap, outap, skap = bufs[inb], bufs[outb], bufs[skipb]

                wtag = f"wide_{name}"
                for _ in range(2):
                    t = actpool.tile([128, Ri * Wo], STREAM_DT, tag=wtag)
                    nc.vector.memset(t[:, :], 0.0)

                for i0 in range(0, w_in, Rg):
                    A0 = (1 + i0) * Wi + 1
                    span = (Rg + 2) * Wi
                    tin = inpool.tile([len(blocks) * cin, (Rg + 2) * Wi],
                                      STREAM_DT, tag="tin_tc")
                    for j, sft in enumerate(blocks):
                        nc.sync.dma_start(
                            out=tin[j * cin:(j + 1) * cin, 0:span],
                            in_=ap_of(inap, A0 + sft,
                                      [[Lpi, cin], [1, span]]))
                    tin_ap = tin[:, :]
                    tpitch = tin_ap.ap[0][0]
                    for py in range(2):
                        for it in range(0, Rg, rows_pt):
                            ps_pair = []
                            for px in range(2):
                                mms = cls[(py, px)]
                                ps = pspool.tile([128, 512], F32, tag="ps")
                                for pk in range(pack):
                                    col = pk * stride
                                    base = (it + pk * Ri) * Wi
                                    for mi, m in enumerate(mms):
                                        K = len(m["segs"]) * cin
                                        rhs = bass.AP(
                                            tin_ap.tensor,
                                            tin_ap.offset + base + m["r"],
                                            [[tpitch, K], [Wi, Ri], [1, w_in]])
                                        nc.tensor.matmul(
                                            ps[col:col + cout, 0:Ri * w_in],
                                            lhsT=wt[f"W_{name}_c{py}{px}_{mi}"][0:K, 0:cout],
                                            rhs=rhs,
                                            start=(mi == 0),
                                            stop=(mi == len(mms) - 1),
                                            tile_position=(0, col))
                                ps_pair.append(ps)
                            wide = actpool.tile([128, Ri * Wo], STREAM_DT,
                                                tag=wtag)
                            wap = wide[:, :]
                            wpitch = wap.ap[0][0]
                            for px in range(2):
                                pap = ps_pair[px][:, :]
                                oap = bass.AP(wap.tensor,
                                              wap.offset + 1 + px,
                                              [[wpitch, 128], [Wo, Ri],
                                               [2, w_in]])
                                iap = bass.AP(pap.tensor, pap.offset,
                                              [[pap.ap[0][0], 128],
                                               [w_in, Ri], [1, w_in]])
                                nc.scalar.activation(oap, iap, AF.Prelu,
                                                     bias=bias, alpha=ALPHA)
                            skt = skpool.tile([128, Ri * Wo], STREAM_DT,
                                              tag="skt")
                            sap = skt[:, :]
                            spitch = sap.ap[0][0]
                            io0 = i0 + it
                            off0 = (1 + 2 * io0 + py) * Wo
                            nc.sync.dma_start(
                                out=bass.AP(sap.tensor, sap.offset,
                                            [[stride * spitch, pack],
                                             [spitch, cout], [Wo, Ri],
                                             [1, Wo]]),
                                in_=ap_of(skap, off0,
                                          [[2 * Ri * Wo, pack], [Lpo, cout],
                                           [2 * Wo, Ri], [1, Wo]]))
                            nc.vector.tensor_add(out=wap, in0=wap, in1=sap)
                            nc.sync.dma_start(
                                out=ap_of(outap, off0,
                                          [[2 * Ri * Wo, pack], [Lpo, cout],
                                           [2 * Wo, Ri], [1, Wo]]),
                                in_=bass.AP(wap.tensor, wap.offset,
                                            [[stride * wpitch, pack],
                                             [wpitch, cout], [Wo, Ri],
                                             [1, Wo]]))

            s12map = {r[0]: r for r in S12}
            tcmap = {r[0]: r for r in TCONVS}
            for li, nm in enumerate(ORDER):
                if nm in s12map:
                    (name, C, Co, h_in, f, s, wsrc, inb, outb) = s12map[nm]
                    emit_s12(li, name, C, Co, h_in, f, s, inb, outb)
                else:
                    (name, cin, cout, w_in, wsrc, inb, outb, skipb) = tcmap[nm]
                    emit_tconv(li, name, cin, cout, w_in, inb, outb, skipb)

            # ---- out layer: 1x1 conv + tri mask --------------------------
            W0 = 514
            Lp0 = (N + 2 + GR) * W0
            d0 = bufs["dec0p"]
            masku = wt["MASKU"][:, :]
            mpitch = masku.ap[0][0]
            bout = wt["B_out"][:, :]
            for ci, t0 in enumerate(range(0, 64, NT)):
                nt = min(NT, 64 - t0)
                span = nt * W0
                tin = inpool.tile([128, NT * W0], STREAM_DT, tag="tinA")
                nc.sync.dma_start(
                    out=tin[0:128, 0:span],
                    in_=ap_of(d0, (1 + 8 * t0) * W0,
                              [[W0, 8], [Lp0, 16], [8 * W0, nt], [1, W0]]))
                tap = tin[:, :]
                tpitch = tap.ap[0][0]
                for bb in range(nt // 4):   # banks of 4 tiles (32 rows)
                    ps = pspool.tile([128, 512], F32, tag="ps")
                    for q in range(4):
                        rhs = bass.AP(tap.tensor,
                                      tap.offset + (bb * 4 + q) * W0 + 1,
                                      [[tpitch, 128], [1, 512]])
                        nc.tensor.matmul(ps[q * 32:q * 32 + 8, 0:512],
                                         lhsT=wt["W_out"][0:128, 0:8], rhs=rhs,
                                         start=True, stop=True,
                                         tile_position=(0, q * 32))
                    a16 = actpool.tile([128, 512], STREAM_DT, tag="a16_out")
                    nc.scalar.activation(a16[:, :], ps[:, :], AF.Identity,
                                         bias=bout)
                    T = (t0 + bb * 4) // 4   # bank index 0..15
                    mslice = bass.AP(masku.tensor, masku.offset + T * 512,
                                     [[mpitch, 128], [1, 512]])
                    nc.vector.tensor_mul(out=a16[:, :], in0=a16[:, :],
                                         in1=mslice)
                    act = actpool.tile([128, 512], F32, tag="act_out")
                    nc.vector.tensor_scalar_add(out=act[:, :], in0=a16[:, :],
                                                scalar1=0.0)
                    for q in range(4):
                        nc.scalar.dma_start(
                            out=ap_of(out_t, (T * 32 + q * 8) * 512,
                                      [[512, 8], [1, 512]]),
                            in_=act[q * 32:q * 32 + 8, 0:512])

            # ---- diagonal softplus patch ---------------------------------
            out_flat = out_t.flatten()
            diag_ap = bass.AP(out_flat.tensor, out_flat.offset, [[513, 512]])
            dt_ = actpool.tile([1, 512], F32, tag="diag")
            nc.sync.dma_start(out=dt_[:, :], in_=diag_ap)
            ta = actpool.tile([1, 512], F32, tag="diag_a")
            nc.scalar.activation(ta[:, :], dt_[:, :], AF.Abs)
            nc.scalar.activation(ta[:, :], ta[:, :], AF.Exp, scale=-1.0)
            nc.vector.tensor_scalar_add(out=ta[:, :], in0=ta[:, :], scalar1=1.0)
            nc.scalar.activation(ta[:, :], ta[:, :], AF.Ln)
            tr = actpool.tile([1, 512], F32, tag="diag_r")
            nc.scalar.activation(tr[:, :], dt_[:, :], AF.Relu)
            nc.vector.tensor_add(out=tr[:, :], in0=tr[:, :], in1=ta[:, :])
            nc.sync.dma_start(out=diag_ap, in_=tr[:, :])

    nc.compile()
    return nc


_NC_CACHE = None


def get_nc():
    global _NC_CACHE
    if _NC_CACHE is None:
        _NC_CACHE = build_unet()
    return _NC_CACHE


def kernel(_trace=False, **inputs):
    nc = get_nc()
    in_maps = make_in_maps(inputs)
    res = run_bass_kernel_spmd(nc, in_maps, core_ids=list(range(B)),
                               trace=_trace)
    out = np.stack([res.results[i]["out"] for i in range(B)], axis=0)
    out = out[:, :, :, None].astype(np.float32)
    if _trace:
        return out, res
    return out
